# revision 2
# baseline (speedup 1.0000x reference)
"""Trainium2 Bass kernel for InteractiveGallingModelV6 batched simulation.

Strategy (tuned via TimelineSim cost-model profiling; ~1.16x the previous
working kernel, 415.8us -> 357.8us simulated per-core):

- Data-parallel over B=65536: 8 cores x 8192 elements, [128 part x 64 free].
- The 150-step recurrence is the whole problem: a single dependency chain is
  latency-bound (~2.7us/step). The batch is split into G=2 independent groups
  of [128 x 32] whose chains the tile scheduler interleaves across engines,
  and each group-step is emitted in two software-pipelined phases
  (S1 = mu-only work + ACT dispatch, S2 = post-ACT work) with half-step skew:
      phase 2k:   S1(g0, k) ; S2(g1, k-1)
      phase 2k+1: S1(g1, k) ; S2(g0, k)
  so every engine FIFO holds ready work while the other group's ACT round
  trip is in flight.
- ACT ops per group-step: Square (completing the square for the sigmoid
  argument), Sigmoid (pi, writes the output slice directly), and ONE wide
  Tanh over a packed [128, 64] tile holding both softplus-fit arguments.
  All three live in the 'sigmoid_and_others' table set (no table switches).
  For |a_mu2| <= 1e-3 the completing-the-square constants blow up, so the
  sigmoid argument falls back to (a_mu2*mu + a_mu)*mu + A0 computed with a
  Pool tensor_scalar + DVE tensor_tensor (both compile-safe op/engine pairs).
- softplus(s0+s_mu*mu+s_T*dT) is approximated as c0 + c2*tanh(a*mu+b) (host
  fit at call time, max fit err ~1e-4; validated end-to-end rel err ~2e-4
  with 1 component flip in 9.8M).
- mu-update pre-adds mu into the branch bases: e_b = s_b*n + ((1+coef)*mu +
  const), so mu' = clip(select(cp, e1, e2)) needs no separate mu+delta add.
  The d1/d2 output channels are then written by per-block BATCHED
  tensor_scalar ops over the stored mu history (15x fixed-cost amortization)
  instead of per-step ops.
- Outputs are staged channel-interleaved [P, K, F, 7] so the output DMA's
  innermost contiguous element is 64*7*4 = 1792B: full DMA rate. (The plain
  per-channel layout's 256B lines run at half rate per the DMA cost model's
  <512B penalty.) The device returns y_dev[t, b, 7]; the host transposes to
  [7, t, b] (pure layout permute).
- Engine assignment tuned empirically (DVE tensor_scalar has a 2x f32 perf
  mode; Pool runs tensor_scalar/tensor_tensor add/mult only -- the backend
  rejects scalar_tensor_tensor and is_ge on Pool).
- Input DMAs for block k+1 are issued before block k's output DMA so the
  in-order SP queue cannot starve the prefetch.
"""
import numpy as np

import concourse.bass as bass
import concourse.bacc as bacc
import concourse.mybir as mybir
from concourse.tile import TileContext
from concourse.bass_utils import run_bass_kernel_spmd

f32 = np.float32
DT = mybir.dt.float32
OP = mybir.AluOpType
AF = mybir.ActivationFunctionType

T_REF = 160.0
MU_MIN, MU_MAX = 0.1, 1.3
N_CYCLES, BATCH = 150, 65536
N_CORES = 8
B_SH = BATCH // N_CORES          # 8192 per core
P = 128
F = B_SH // P                    # 64
G = 2                            # pipelined groups per core
FG = F // G                      # 32
K_BLK = 10                       # steps per DMA block (150 % 10 == 0)
NCH = 7

PARAM_NAMES = ['a0', 'a_T', 'a_mu', 'a_mu2', 'c0', 'c_mu', 'c_T', 's0', 's_mu', 's_T',
               'j0', 'j_mu', 'j_T', 'v0', 'v_mu', 'mu0_base', 'mu0_T']


def _softplus64(x):
    return np.logaddexp(0.0, x)


def _fit_tanh_model(mu_grid, f_vals):
    """Fit f(mu) ~= c0 + c2*tanh(a*mu + b) (coarse-to-fine in (a,b), lstsq
    for the linear coefficients). Returns (a, b, c0, c2)."""
    best = None
    a_grid = np.linspace(0.1, 5.0, 60)
    b_grid = np.linspace(-5.0, 5.0, 101)
    ones = np.ones_like(mu_grid)
    for _ in range(5):
        for a in a_grid:
            for b in b_grid:
                t = np.tanh(a * mu_grid + b)
                A = np.stack([ones, t], 1)
                c, *_ = np.linalg.lstsq(A, f_vals, rcond=None)
                err = np.max(np.abs(A @ c - f_vals))
                if best is None or err < best[0]:
                    best = (err, a, b, c)
        _, a0_, b0_, _ = best
        da = a_grid[1] - a_grid[0]
        db = b_grid[1] - b_grid[0]
        a_grid = np.linspace(a0_ - da, a0_ + da, 21)
        b_grid = np.linspace(b0_ - db, b0_ + db, 21)
    _, a, b, c = best
    return float(a), float(b), float(c[0]), float(c[1])


def _prep_consts(params, T):
    p = {n: float(params[i]) for i, n in enumerate(PARAM_NAMES)}
    dT = float(T) - T_REF
    a_mu2 = p['a_mu2']
    A0 = p['a0'] + p['a_T'] * dT
    mu_grid = np.linspace(MU_MIN, MU_MAX, 4001)
    a1, b1, c01, c21 = _fit_tanh_model(
        mu_grid, _softplus64(p['s0'] + p['s_mu'] * mu_grid + p['s_T'] * dT))
    a2, b2, c02, c22 = _fit_tanh_model(
        mu_grid, _softplus64(p['v0'] + p['v_mu'] * mu_grid))
    D1b = p['c0'] + p['c_T'] * dT
    D2b = p['j0'] + p['j_T'] * dT
    mu0 = float(np.clip(np.float32(p['mu0_base']) + np.float32(p['mu0_T'] * dT),
                        MU_MIN, MU_MAX))
    return (p['a_mu'], a_mu2, A0, a1, b1, c01, c21, a2, b2, c02, c22,
            p['c_mu'], D1b, p['j_mu'], D2b, mu0)


def _build_nc(consts):
    (a_mu, a_mu2, A0, a1, b1, c01, c21, a2, b2, c02, c22,
     c_mu, D1b, j_mu, D2b, mu0) = [float(v) for v in consts]

    # completing-the-square constants for pi = sigmoid(a_mu2*(mu+h)^2 + k);
    # fall back to the split affine*mu form when a_mu2 is too small for the
    # cancellation in k_cs to stay accurate in f32.
    use_quad_act = abs(a_mu2) > 1e-3
    if use_quad_act:
        h_cs = a_mu / (2.0 * a_mu2)
        k_cs = A0 - a_mu2 * h_cs * h_cs
    else:
        h_cs = k_cs = 0.0

    nc = bacc.Bacc("TRN2", target_bir_lowering=False)
    u_d = nc.declare_dram_parameter("u", [N_CYCLES, B_SH], DT, isOutput=False)
    n_d = nc.declare_dram_parameter("noise", [N_CYCLES, B_SH], DT, isOutput=False)
    y_d = nc.declare_dram_parameter("y", [N_CYCLES, B_SH, NCH], DT, isOutput=True)

    u_v = u_d[:].rearrange("t (p f) -> p t f", p=P)
    n_v = n_d[:].rearrange("t (p f) -> p t f", p=P)
    y_v = y_d[:].rearrange("t (p f) j -> p t f j", p=P)

    NBLK = N_CYCLES // K_BLK

    with TileContext(nc) as tc:
        with (
            tc.tile_pool(name="io", bufs=2) as io_pool,
            tc.tile_pool(name="tmp", bufs=3) as tmp_pool,
            tc.tile_pool(name="state", bufs=1) as st_pool,
        ):
            mu_init = st_pool.tile([P, F], DT)
            nc.vector.memset(mu_init[:], mu0)

            biases = st_pool.tile([P, 4], DT)
            nc.vector.memset(biases[:, 0:1], A0)
            nc.vector.memset(biases[:, 1:2], 0.0)
            nc.vector.memset(biases[:, 2:3], h_cs)
            nc.vector.memset(biases[:, 3:4], k_cs)
            A0_ap = biases[:, 0:1]
            zero_ap = biases[:, 1:2]
            h_ap = biases[:, 2:3]
            k_ap = biases[:, 3:4]

            mu_g = [mu_init[:, g * FG:(g + 1) * FG] for g in range(G)]
            st = [None, None]
            tu = [None] * NBLK
            tn = [None] * NBLK
            yt = [None] * NBLK

            def issue_in(blk):
                t0 = blk * K_BLK
                tu[blk] = io_pool.tile([P, K_BLK, F], DT, tag="u", name=f"u{blk}")
                tn[blk] = io_pool.tile([P, K_BLK, F], DT, tag="n", name=f"n{blk}")
                nc.sync.dma_start(out=tu[blk][:], in_=u_v[:, t0:t0 + K_BLK, :])
                nc.sync.dma_start(out=tn[blk][:], in_=n_v[:, t0:t0 + K_BLK, :])

            def S1(g, k):
                """mu-only stage: sigmoid-arg, tanh pack, ACT dispatch."""
                blk = k // K_BLK
                ki = k % K_BLK
                lo, hi = g * FG, (g + 1) * FG
                mu = mu_g[g]
                ytile = yt[blk]
                q = tmp_pool.tile([P, FG], DT, tag=f"q{g}", name=f"q{g}_{k}")
                z = tmp_pool.tile([P, 2 * FG], DT, tag=f"z{g}", name=f"z{g}_{k}")
                nc.vector.tensor_scalar(z[:, 0:FG], mu, a1, b1, OP.mult, OP.add)
                nc.vector.tensor_scalar(z[:, FG:2 * FG], mu, a2, b2,
                                        OP.mult, OP.add)
                if use_quad_act:
                    nc.scalar.activation(q[:], mu, AF.Square, bias=h_ap,
                                         scale=1.0)
                else:
                    qv = tmp_pool.tile([P, FG], DT, tag=f"qv{g}",
                                       name=f"qv{g}_{k}")
                    nc.gpsimd.tensor_scalar(qv[:], mu, a_mu2, a_mu,
                                            OP.mult, OP.add)
                    nc.vector.tensor_tensor(q[:], qv[:], mu, OP.mult)
                Tt_tile = tmp_pool.tile([P, 2 * FG], DT, tag=f"T{g}",
                                        name=f"T{g}_{k}")
                Tt = Tt_tile[:]
                nc.scalar.activation(Tt, z[:], AF.Tanh, bias=zero_ap, scale=1.0)
                if use_quad_act:
                    nc.scalar.activation(ytile[:, ki, lo:hi, 2], q[:],
                                         AF.Sigmoid, bias=k_ap, scale=a_mu2)
                else:
                    nc.scalar.activation(ytile[:, ki, lo:hi, 2], q[:],
                                         AF.Sigmoid, bias=A0_ap, scale=1.0)
                st[g] = (q, Tt)

            def S2(g, k):
                """post-ACT stage: cp, sigmas, branches, select, clip."""
                blk = k // K_BLK
                ki = k % K_BLK
                lo, hi = g * FG, (g + 1) * FG
                mu = mu_g[g]
                ytile = yt[blk]
                _, Tt = st[g]
                T1 = Tt[:, 0:FG]
                T2 = Tt[:, FG:2 * FG]
                u_s = tu[blk][:, ki, lo:hi]
                n_s = tn[blk][:, ki, lo:hi]
                o_mu = ytile[:, ki, lo:hi, 0]
                o_cp = ytile[:, ki, lo:hi, 1]
                o_pi = ytile[:, ki, lo:hi, 2]
                o_s1 = ytile[:, ki, lo:hi, 4]
                o_s2 = ytile[:, ki, lo:hi, 6]

                m1 = tmp_pool.tile([P, FG], DT, tag=f"m1{g}", name=f"m1{g}_{k}")
                m2 = tmp_pool.tile([P, FG], DT, tag=f"m2{g}", name=f"m2{g}_{k}")
                e1 = tmp_pool.tile([P, FG], DT, tag=f"e1{g}", name=f"e1{g}_{k}")
                e2 = tmp_pool.tile([P, FG], DT, tag=f"e2{g}", name=f"e2{g}_{k}")
                D1m = tmp_pool.tile([P, FG], DT, tag=f"D1m{g}", name=f"D1m{g}_{k}")
                D2m = tmp_pool.tile([P, FG], DT, tag=f"D2m{g}", name=f"D2m{g}_{k}")

                nc.vector.tensor_tensor(o_cp, u_s, o_pi, OP.is_ge)
                nc.gpsimd.tensor_scalar(o_s1, T1, c21, c01, OP.mult, OP.add)
                nc.gpsimd.tensor_scalar(o_s2, T2, c22, c02, OP.mult, OP.add)
                nc.vector.tensor_tensor(m1[:], o_s1, n_s, OP.mult)
                nc.gpsimd.tensor_tensor(m2[:], o_s2, n_s, OP.mult)
                # pre-add mu into the branch bases; d outputs are batched later
                nc.gpsimd.tensor_scalar(D1m[:], mu, 1.0 + c_mu, D1b,
                                        OP.mult, OP.add)
                nc.gpsimd.tensor_scalar(D2m[:], mu, 1.0 + j_mu, D2b,
                                        OP.mult, OP.add)
                nc.vector.tensor_tensor(e1[:], m1[:], D1m[:], OP.add)
                nc.gpsimd.tensor_tensor(e2[:], m2[:], D2m[:], OP.add)
                nc.vector.copy_predicated(
                    e1[:], o_cp.bitcast(mybir.dt.uint32), e2[:])
                nc.vector.tensor_scalar(o_mu, e1[:], MU_MIN, MU_MAX,
                                        OP.max, OP.min)
                mu_g[g] = o_mu

            def finish_block(blk):
                """batched d1/d2 writeback over stored mu history + out DMA."""
                t0 = blk * K_BLK
                yb = yt[blk]
                if K_BLK > 1:
                    nc.vector.tensor_scalar(yb[:, 1:K_BLK, :, 3],
                                            yb[:, 0:K_BLK - 1, :, 0],
                                            c_mu, D1b, OP.mult, OP.add)
                    nc.vector.tensor_scalar(yb[:, 1:K_BLK, :, 5],
                                            yb[:, 0:K_BLK - 1, :, 0],
                                            j_mu, D2b, OP.mult, OP.add)
                if blk == 0:
                    mu_prev = mu_init[:]
                else:
                    mu_prev = yt[blk - 1][:, K_BLK - 1, :, 0]
                nc.vector.tensor_scalar(yb[:, 0, :, 3], mu_prev,
                                        c_mu, D1b, OP.mult, OP.add)
                nc.vector.tensor_scalar(yb[:, 0, :, 5], mu_prev,
                                        j_mu, D2b, OP.mult, OP.add)
                nc.sync.dma_start(out=y_v[:, t0:t0 + K_BLK, :, :], in_=yb[:])

            issue_in(0)
            for k in range(N_CYCLES):
                blk = k // K_BLK
                if k % K_BLK == 0:
                    yt[blk] = io_pool.tile([P, K_BLK, F, NCH], DT, tag="y",
                                           name=f"y{blk}")
                    if blk + 1 < NBLK:
                        issue_in(blk + 1)
                S1(0, k)
                if k > 0:
                    S2(1, k - 1)
                    if k % K_BLK == 0:
                        finish_block(blk - 1)
                S1(1, k)
                S2(0, k)
            S2(1, N_CYCLES - 1)
            finish_block(NBLK - 1)

    return nc


_CACHE = {}


def _get_nc(consts):
    key = tuple(np.float64(consts).tobytes())
    if key not in _CACHE:
        nc = _build_nc(consts)
        nc.finalize()
        _CACHE[key] = nc
    return _CACHE[key]


def kernel(params, T, u, noise):
    params = np.asarray(params, dtype=np.float32)
    u = np.ascontiguousarray(np.asarray(u, dtype=np.float32))
    noise = np.ascontiguousarray(np.asarray(noise, dtype=np.float32))
    consts = _prep_consts(params, float(np.asarray(T)))
    nc = _get_nc(consts)

    in_maps = []
    for c in range(N_CORES):
        sl = slice(c * B_SH, (c + 1) * B_SH)
        in_maps.append({
            "u": np.ascontiguousarray(u[:, sl]),
            "noise": np.ascontiguousarray(noise[:, sl]),
        })
    res = run_bass_kernel_spmd(nc, in_maps, list(range(N_CORES)))
    out = np.empty((NCH, N_CYCLES, BATCH), np.float32)
    for c in range(N_CORES):
        sl = slice(c * B_SH, (c + 1) * B_SH)
        out[:, :, sl] = res.results[c]["y"].transpose(2, 0, 1)
    return out


if __name__ == "__main__":
    rng = np.random.default_rng(0)
    params = np.array([2.0, -0.1, -1.0, 0.5, 0.01, -0.02, 0.001, -3.0, 1.0, 0.1,
                       0.5, -1.0, 0.02, -1.5, 0.5, 0.12, 0.005], np.float32)
    u = rng.random((N_CYCLES, BATCH), dtype=np.float32)
    noise = rng.standard_normal((N_CYCLES, BATCH), dtype=np.float32)
    y = kernel(params=params, T=np.float32(200.0), u=u, noise=noise)
    print("out", y.shape, y.dtype, float(y[0].mean()))


# revision 3
# speedup vs baseline: 1.0495x; 1.0495x over previous
"""Trainium2 Bass kernel for InteractiveGallingModelV6 batched simulation.

Strategy (tuned via TimelineSim cost-model profiling; ~1.16x the previous
working kernel, 415.8us -> 357.8us simulated per-core):

- Data-parallel over B=65536: 8 cores x 8192 elements, [128 part x 64 free].
- The 150-step recurrence is the whole problem: a single dependency chain is
  latency-bound (~2.7us/step). The batch is split into G=2 independent groups
  of [128 x 32] whose chains the tile scheduler interleaves across engines,
  and each group-step is emitted in two software-pipelined phases
  (S1 = mu-only work + ACT dispatch, S2 = post-ACT work) with half-step skew:
      phase 2k:   S1(g0, k) ; S2(g1, k-1)
      phase 2k+1: S1(g1, k) ; S2(g0, k)
  so every engine FIFO holds ready work while the other group's ACT round
  trip is in flight.
- ACT ops per group-step: Square (completing the square for the sigmoid
  argument), Sigmoid (pi, writes the output slice directly), and ONE wide
  Tanh over a packed [128, 64] tile holding both softplus-fit arguments.
  All three live in the 'sigmoid_and_others' table set (no table switches).
  For |a_mu2| <= 1e-3 the completing-the-square constants blow up, so the
  sigmoid argument falls back to (a_mu2*mu + a_mu)*mu + A0 computed with a
  Pool tensor_scalar + DVE tensor_tensor (both compile-safe op/engine pairs).
- softplus(s0+s_mu*mu+s_T*dT) is approximated as c0 + c2*tanh(a*mu+b) (host
  fit at call time, max fit err ~1e-4; validated end-to-end rel err ~2e-4
  with 1 component flip in 9.8M).
- mu-update pre-adds mu into the branch bases: e_b = s_b*n + ((1+coef)*mu +
  const), so mu' = clip(select(cp, e1, e2)) needs no separate mu+delta add.
  The d1/d2 output channels are then written by per-block BATCHED
  tensor_scalar ops over the stored mu history (15x fixed-cost amortization)
  instead of per-step ops.
- Outputs are staged channel-interleaved [P, K, F, 7] so the output DMA's
  innermost contiguous element is 64*7*4 = 1792B: full DMA rate. (The plain
  per-channel layout's 256B lines run at half rate per the DMA cost model's
  <512B penalty.) The device returns y_dev[t, b, 7]; the host transposes to
  [7, t, b] (pure layout permute).
- Engine assignment tuned empirically (DVE tensor_scalar has a 2x f32 perf
  mode; Pool runs tensor_scalar/tensor_tensor add/mult only -- the backend
  rejects scalar_tensor_tensor and is_ge on Pool).
- Input DMAs for block k+1 are issued before block k's output DMA so the
  in-order SP queue cannot starve the prefetch.
"""
import numpy as np

import concourse.bass as bass
import concourse.bacc as bacc
import concourse.mybir as mybir
from concourse.tile import TileContext
from concourse.bass_utils import run_bass_kernel_spmd

f32 = np.float32
DT = mybir.dt.float32
OP = mybir.AluOpType
AF = mybir.ActivationFunctionType

T_REF = 160.0
MU_MIN, MU_MAX = 0.1, 1.3
N_CYCLES, BATCH = 150, 65536
N_CORES = 8
B_SH = BATCH // N_CORES          # 8192 per core
P = 128
F = B_SH // P                    # 64
G = 2                            # pipelined groups per core
FG = F // G                      # 32
K_BLK = 10                       # steps per DMA block (150 % 10 == 0)
NCH = 7

PARAM_NAMES = ['a0', 'a_T', 'a_mu', 'a_mu2', 'c0', 'c_mu', 'c_T', 's0', 's_mu', 's_T',
               'j0', 'j_mu', 'j_T', 'v0', 'v_mu', 'mu0_base', 'mu0_T']


def _softplus64(x):
    return np.logaddexp(0.0, x)


def _fit_tanh_model(mu_grid, f_vals):
    """Fit f(mu) ~= c0 + c2*tanh(a*mu + b) (coarse-to-fine in (a,b), lstsq
    for the linear coefficients). Returns (a, b, c0, c2)."""
    best = None
    a_grid = np.linspace(0.1, 5.0, 60)
    b_grid = np.linspace(-5.0, 5.0, 101)
    ones = np.ones_like(mu_grid)
    for _ in range(5):
        for a in a_grid:
            for b in b_grid:
                t = np.tanh(a * mu_grid + b)
                A = np.stack([ones, t], 1)
                c, *_ = np.linalg.lstsq(A, f_vals, rcond=None)
                err = np.max(np.abs(A @ c - f_vals))
                if best is None or err < best[0]:
                    best = (err, a, b, c)
        _, a0_, b0_, _ = best
        da = a_grid[1] - a_grid[0]
        db = b_grid[1] - b_grid[0]
        a_grid = np.linspace(a0_ - da, a0_ + da, 21)
        b_grid = np.linspace(b0_ - db, b0_ + db, 21)
    _, a, b, c = best
    return float(a), float(b), float(c[0]), float(c[1])


def _prep_consts(params, T):
    p = {n: float(params[i]) for i, n in enumerate(PARAM_NAMES)}
    dT = float(T) - T_REF
    a_mu2 = p['a_mu2']
    A0 = p['a0'] + p['a_T'] * dT
    mu_grid = np.linspace(MU_MIN, MU_MAX, 4001)
    a1, b1, c01, c21 = _fit_tanh_model(
        mu_grid, _softplus64(p['s0'] + p['s_mu'] * mu_grid + p['s_T'] * dT))
    a2, b2, c02, c22 = _fit_tanh_model(
        mu_grid, _softplus64(p['v0'] + p['v_mu'] * mu_grid))
    D1b = p['c0'] + p['c_T'] * dT
    D2b = p['j0'] + p['j_T'] * dT
    mu0 = float(np.clip(np.float32(p['mu0_base']) + np.float32(p['mu0_T'] * dT),
                        MU_MIN, MU_MAX))
    return (p['a_mu'], a_mu2, A0, a1, b1, c01, c21, a2, b2, c02, c22,
            p['c_mu'], D1b, p['j_mu'], D2b, mu0)


def _build_nc(consts):
    (a_mu, a_mu2, A0, a1, b1, c01, c21, a2, b2, c02, c22,
     c_mu, D1b, j_mu, D2b, mu0) = [float(v) for v in consts]

    # completing-the-square constants for pi = sigmoid(a_mu2*(mu+h)^2 + k);
    # fall back to the split affine*mu form when a_mu2 is too small for the
    # cancellation in k_cs to stay accurate in f32.
    use_quad_act = abs(a_mu2) > 1e-3
    if use_quad_act:
        h_cs = a_mu / (2.0 * a_mu2)
        k_cs = A0 - a_mu2 * h_cs * h_cs
    else:
        h_cs = k_cs = 0.0

    nc = bacc.Bacc("TRN2", target_bir_lowering=False)
    u_d = nc.declare_dram_parameter("u", [N_CYCLES, B_SH], DT, isOutput=False)
    n_d = nc.declare_dram_parameter("noise", [N_CYCLES, B_SH], DT, isOutput=False)
    y_d = nc.declare_dram_parameter("y", [N_CYCLES, B_SH, NCH], DT, isOutput=True)

    u_v = u_d[:].rearrange("t (p f) -> p t f", p=P)
    n_v = n_d[:].rearrange("t (p f) -> p t f", p=P)
    y_v = y_d[:].rearrange("t (p f) j -> p t f j", p=P)

    NBLK = N_CYCLES // K_BLK

    with TileContext(nc) as tc:
        with (
            tc.tile_pool(name="io", bufs=2) as io_pool,
            tc.tile_pool(name="tmp", bufs=3) as tmp_pool,
            tc.tile_pool(name="state", bufs=1) as st_pool,
        ):
            mu_init = st_pool.tile([P, F], DT)
            nc.vector.memset(mu_init[:], mu0)

            biases = st_pool.tile([P, 4], DT)
            nc.vector.memset(biases[:, 0:1], A0)
            nc.vector.memset(biases[:, 1:2], 0.0)
            nc.vector.memset(biases[:, 2:3], h_cs)
            nc.vector.memset(biases[:, 3:4], k_cs)
            A0_ap = biases[:, 0:1]
            zero_ap = biases[:, 1:2]
            h_ap = biases[:, 2:3]
            k_ap = biases[:, 3:4]

            mu_g = [mu_init[:, g * FG:(g + 1) * FG] for g in range(G)]
            st = [None, None]
            tu = [None] * NBLK
            tn = [None] * NBLK
            yt = [None] * NBLK

            def issue_in(blk):
                t0 = blk * K_BLK
                tu[blk] = io_pool.tile([P, K_BLK, F], DT, tag="u", name=f"u{blk}")
                tn[blk] = io_pool.tile([P, K_BLK, F], DT, tag="n", name=f"n{blk}")
                nc.sync.dma_start(out=tu[blk][:], in_=u_v[:, t0:t0 + K_BLK, :])
                nc.sync.dma_start(out=tn[blk][:], in_=n_v[:, t0:t0 + K_BLK, :])

            def S1(g, k):
                """mu-only stage: sigmoid-arg, tanh pack, ACT dispatch."""
                blk = k // K_BLK
                ki = k % K_BLK
                lo, hi = g * FG, (g + 1) * FG
                mu = mu_g[g]
                ytile = yt[blk]
                q = tmp_pool.tile([P, FG], DT, tag=f"q{g}", name=f"q{g}_{k}")
                z = tmp_pool.tile([P, 2 * FG], DT, tag=f"z{g}", name=f"z{g}_{k}")
                nc.vector.tensor_scalar(z[:, 0:FG], mu, a1, b1, OP.mult, OP.add)
                nc.vector.tensor_scalar(z[:, FG:2 * FG], mu, a2, b2,
                                        OP.mult, OP.add)
                if use_quad_act:
                    nc.scalar.activation(q[:], mu, AF.Square, bias=h_ap,
                                         scale=1.0)
                else:
                    qv = tmp_pool.tile([P, FG], DT, tag=f"qv{g}",
                                       name=f"qv{g}_{k}")
                    nc.gpsimd.tensor_scalar(qv[:], mu, a_mu2, a_mu,
                                            OP.mult, OP.add)
                    nc.vector.tensor_tensor(q[:], qv[:], mu, OP.mult)
                Tt_tile = tmp_pool.tile([P, 2 * FG], DT, tag=f"T{g}",
                                        name=f"T{g}_{k}")
                Tt = Tt_tile[:]
                nc.scalar.activation(Tt, z[:], AF.Tanh, bias=zero_ap, scale=1.0)
                if use_quad_act:
                    nc.scalar.activation(ytile[:, ki, lo:hi, 2], q[:],
                                         AF.Sigmoid, bias=k_ap, scale=a_mu2)
                else:
                    nc.scalar.activation(ytile[:, ki, lo:hi, 2], q[:],
                                         AF.Sigmoid, bias=A0_ap, scale=1.0)
                st[g] = (q, Tt)

            def S2(g, k):
                """post-ACT stage: cp, sigmas, branches, select, clip."""
                blk = k // K_BLK
                ki = k % K_BLK
                lo, hi = g * FG, (g + 1) * FG
                mu = mu_g[g]
                ytile = yt[blk]
                _, Tt = st[g]
                T1 = Tt[:, 0:FG]
                T2 = Tt[:, FG:2 * FG]
                u_s = tu[blk][:, ki, lo:hi]
                n_s = tn[blk][:, ki, lo:hi]
                o_mu = ytile[:, ki, lo:hi, 0]
                o_cp = ytile[:, ki, lo:hi, 1]
                o_pi = ytile[:, ki, lo:hi, 2]
                o_s1 = ytile[:, ki, lo:hi, 4]
                o_s2 = ytile[:, ki, lo:hi, 6]

                m1 = tmp_pool.tile([P, FG], DT, tag=f"m1{g}", name=f"m1{g}_{k}")
                m2 = tmp_pool.tile([P, FG], DT, tag=f"m2{g}", name=f"m2{g}_{k}")
                e1 = tmp_pool.tile([P, FG], DT, tag=f"e1{g}", name=f"e1{g}_{k}")
                e2 = tmp_pool.tile([P, FG], DT, tag=f"e2{g}", name=f"e2{g}_{k}")
                D1m = tmp_pool.tile([P, FG], DT, tag=f"D1m{g}", name=f"D1m{g}_{k}")
                D2m = tmp_pool.tile([P, FG], DT, tag=f"D2m{g}", name=f"D2m{g}_{k}")

                nc.vector.tensor_tensor(o_cp, u_s, o_pi, OP.is_ge)
                nc.vector.tensor_scalar(o_s1, T1, c21, c01, OP.mult, OP.add)
                nc.gpsimd.tensor_scalar(o_s2, T2, c22, c02, OP.mult, OP.add)
                nc.vector.tensor_tensor(m1[:], o_s1, n_s, OP.mult)
                nc.gpsimd.tensor_tensor(m2[:], o_s2, n_s, OP.mult)
                # pre-add mu into the branch bases; d outputs are batched later
                nc.gpsimd.tensor_scalar(D1m[:], mu, 1.0 + c_mu, D1b,
                                        OP.mult, OP.add)
                nc.gpsimd.tensor_scalar(D2m[:], mu, 1.0 + j_mu, D2b,
                                        OP.mult, OP.add)
                nc.vector.tensor_tensor(e1[:], m1[:], D1m[:], OP.add)
                nc.gpsimd.tensor_tensor(e2[:], m2[:], D2m[:], OP.add)
                nc.vector.copy_predicated(
                    e1[:], o_cp.bitcast(mybir.dt.uint32), e2[:])
                nc.vector.tensor_scalar(o_mu, e1[:], MU_MIN, MU_MAX,
                                        OP.max, OP.min)
                mu_g[g] = o_mu

            def finish_block(blk):
                """batched d1/d2 writeback over stored mu history + out DMA."""
                t0 = blk * K_BLK
                yb = yt[blk]
                if K_BLK > 1:
                    nc.vector.tensor_scalar(yb[:, 1:K_BLK, :, 3],
                                            yb[:, 0:K_BLK - 1, :, 0],
                                            c_mu, D1b, OP.mult, OP.add)
                    nc.vector.tensor_scalar(yb[:, 1:K_BLK, :, 5],
                                            yb[:, 0:K_BLK - 1, :, 0],
                                            j_mu, D2b, OP.mult, OP.add)
                if blk == 0:
                    mu_prev = mu_init[:]
                else:
                    mu_prev = yt[blk - 1][:, K_BLK - 1, :, 0]
                nc.vector.tensor_scalar(yb[:, 0, :, 3], mu_prev,
                                        c_mu, D1b, OP.mult, OP.add)
                nc.vector.tensor_scalar(yb[:, 0, :, 5], mu_prev,
                                        j_mu, D2b, OP.mult, OP.add)
                nc.sync.dma_start(out=y_v[:, t0:t0 + K_BLK, :, :], in_=yb[:])

            issue_in(0)
            for k in range(N_CYCLES):
                blk = k // K_BLK
                if k % K_BLK == 0:
                    yt[blk] = io_pool.tile([P, K_BLK, F, NCH], DT, tag="y",
                                           name=f"y{blk}")
                    if blk + 1 < NBLK:
                        issue_in(blk + 1)
                S1(0, k)
                if k > 0:
                    S2(1, k - 1)
                    if k % K_BLK == 0:
                        finish_block(blk - 1)
                S1(1, k)
                S2(0, k)
            S2(1, N_CYCLES - 1)
            finish_block(NBLK - 1)

    return nc


_CACHE = {}


def _get_nc(consts):
    key = tuple(np.float64(consts).tobytes())
    if key not in _CACHE:
        nc = _build_nc(consts)
        nc.finalize()
        _CACHE[key] = nc
    return _CACHE[key]


def kernel(params, T, u, noise):
    params = np.asarray(params, dtype=np.float32)
    u = np.ascontiguousarray(np.asarray(u, dtype=np.float32))
    noise = np.ascontiguousarray(np.asarray(noise, dtype=np.float32))
    consts = _prep_consts(params, float(np.asarray(T)))
    nc = _get_nc(consts)

    in_maps = []
    for c in range(N_CORES):
        sl = slice(c * B_SH, (c + 1) * B_SH)
        in_maps.append({
            "u": np.ascontiguousarray(u[:, sl]),
            "noise": np.ascontiguousarray(noise[:, sl]),
        })
    res = run_bass_kernel_spmd(nc, in_maps, list(range(N_CORES)))
    out = np.empty((NCH, N_CYCLES, BATCH), np.float32)
    for c in range(N_CORES):
        sl = slice(c * B_SH, (c + 1) * B_SH)
        out[:, :, sl] = res.results[c]["y"].transpose(2, 0, 1)
    return out


if __name__ == "__main__":
    rng = np.random.default_rng(0)
    params = np.array([2.0, -0.1, -1.0, 0.5, 0.01, -0.02, 0.001, -3.0, 1.0, 0.1,
                       0.5, -1.0, 0.02, -1.5, 0.5, 0.12, 0.005], np.float32)
    u = rng.random((N_CYCLES, BATCH), dtype=np.float32)
    noise = rng.standard_normal((N_CYCLES, BATCH), dtype=np.float32)
    y = kernel(params=params, T=np.float32(200.0), u=u, noise=noise)
    print("out", y.shape, y.dtype, float(y[0].mean()))


# revision 4
# speedup vs baseline: 1.0576x; 1.0077x over previous
"""Trainium2 Bass kernel for InteractiveGallingModelV6 batched simulation.

Strategy (tuned via TimelineSim cost-model profiling; ~1.16x the previous
working kernel, 415.8us -> 357.8us simulated per-core):

- Data-parallel over B=65536: 8 cores x 8192 elements, [128 part x 64 free].
- The 150-step recurrence is the whole problem: a single dependency chain is
  latency-bound (~2.7us/step). The batch is split into G=2 independent groups
  of [128 x 32] whose chains the tile scheduler interleaves across engines,
  and each group-step is emitted in two software-pipelined phases
  (S1 = mu-only work + ACT dispatch, S2 = post-ACT work) with half-step skew:
      phase 2k:   S1(g0, k) ; S2(g1, k-1)
      phase 2k+1: S1(g1, k) ; S2(g0, k)
  so every engine FIFO holds ready work while the other group's ACT round
  trip is in flight.
- ACT ops per group-step: Square (completing the square for the sigmoid
  argument), Sigmoid (pi, writes the output slice directly), and ONE wide
  Tanh over a packed [128, 64] tile holding both softplus-fit arguments.
  All three live in the 'sigmoid_and_others' table set (no table switches).
  For |a_mu2| <= 1e-3 the completing-the-square constants blow up, so the
  sigmoid argument falls back to (a_mu2*mu + a_mu)*mu + A0 computed with a
  Pool tensor_scalar + DVE tensor_tensor (both compile-safe op/engine pairs).
- softplus(s0+s_mu*mu+s_T*dT) is approximated as c0 + c2*tanh(a*mu+b) (host
  fit at call time, max fit err ~1e-4; validated end-to-end rel err ~2e-4
  with 1 component flip in 9.8M).
- mu-update pre-adds mu into the branch bases: e_b = s_b*n + ((1+coef)*mu +
  const), so mu' = clip(select(cp, e1, e2)) needs no separate mu+delta add.
  The d1/d2 output channels are then written by per-block BATCHED
  tensor_scalar ops over the stored mu history (15x fixed-cost amortization)
  instead of per-step ops.
- Outputs are staged channel-interleaved [P, K, F, 7] so the output DMA's
  innermost contiguous element is 64*7*4 = 1792B: full DMA rate. (The plain
  per-channel layout's 256B lines run at half rate per the DMA cost model's
  <512B penalty.) The device returns y_dev[t, b, 7]; the host transposes to
  [7, t, b] (pure layout permute).
- Engine assignment tuned empirically (DVE tensor_scalar has a 2x f32 perf
  mode; Pool runs tensor_scalar/tensor_tensor add/mult only -- the backend
  rejects scalar_tensor_tensor and is_ge on Pool).
- Input DMAs for block k+1 are issued before block k's output DMA so the
  in-order SP queue cannot starve the prefetch.
"""
import numpy as np

import concourse.bass as bass
import concourse.bacc as bacc
import concourse.mybir as mybir
from concourse.tile import TileContext
from concourse.bass_utils import run_bass_kernel_spmd

f32 = np.float32
DT = mybir.dt.float32
OP = mybir.AluOpType
AF = mybir.ActivationFunctionType

T_REF = 160.0
MU_MIN, MU_MAX = 0.1, 1.3
N_CYCLES, BATCH = 150, 65536
N_CORES = 8
B_SH = BATCH // N_CORES          # 8192 per core
P = 128
F = B_SH // P                    # 64
G = 2                            # pipelined groups per core
FG = F // G                      # 32
K_BLK = 10                       # steps per DMA block (150 % 10 == 0)
NCH = 7

PARAM_NAMES = ['a0', 'a_T', 'a_mu', 'a_mu2', 'c0', 'c_mu', 'c_T', 's0', 's_mu', 's_T',
               'j0', 'j_mu', 'j_T', 'v0', 'v_mu', 'mu0_base', 'mu0_T']


def _softplus64(x):
    return np.logaddexp(0.0, x)


def _fit_tanh_model(mu_grid, f_vals):
    """Fit f(mu) ~= c0 + c2*tanh(a*mu + b) (coarse-to-fine in (a,b), lstsq
    for the linear coefficients). Returns (a, b, c0, c2)."""
    best = None
    a_grid = np.linspace(0.1, 5.0, 60)
    b_grid = np.linspace(-5.0, 5.0, 101)
    ones = np.ones_like(mu_grid)
    for _ in range(5):
        for a in a_grid:
            for b in b_grid:
                t = np.tanh(a * mu_grid + b)
                A = np.stack([ones, t], 1)
                c, *_ = np.linalg.lstsq(A, f_vals, rcond=None)
                err = np.max(np.abs(A @ c - f_vals))
                if best is None or err < best[0]:
                    best = (err, a, b, c)
        _, a0_, b0_, _ = best
        da = a_grid[1] - a_grid[0]
        db = b_grid[1] - b_grid[0]
        a_grid = np.linspace(a0_ - da, a0_ + da, 21)
        b_grid = np.linspace(b0_ - db, b0_ + db, 21)
    _, a, b, c = best
    return float(a), float(b), float(c[0]), float(c[1])


def _prep_consts(params, T):
    p = {n: float(params[i]) for i, n in enumerate(PARAM_NAMES)}
    dT = float(T) - T_REF
    a_mu2 = p['a_mu2']
    A0 = p['a0'] + p['a_T'] * dT
    mu_grid = np.linspace(MU_MIN, MU_MAX, 4001)
    a1, b1, c01, c21 = _fit_tanh_model(
        mu_grid, _softplus64(p['s0'] + p['s_mu'] * mu_grid + p['s_T'] * dT))
    a2, b2, c02, c22 = _fit_tanh_model(
        mu_grid, _softplus64(p['v0'] + p['v_mu'] * mu_grid))
    D1b = p['c0'] + p['c_T'] * dT
    D2b = p['j0'] + p['j_T'] * dT
    mu0 = float(np.clip(np.float32(p['mu0_base']) + np.float32(p['mu0_T'] * dT),
                        MU_MIN, MU_MAX))
    return (p['a_mu'], a_mu2, A0, a1, b1, c01, c21, a2, b2, c02, c22,
            p['c_mu'], D1b, p['j_mu'], D2b, mu0)


def _build_nc(consts):
    (a_mu, a_mu2, A0, a1, b1, c01, c21, a2, b2, c02, c22,
     c_mu, D1b, j_mu, D2b, mu0) = [float(v) for v in consts]

    # completing-the-square constants for pi = sigmoid(a_mu2*(mu+h)^2 + k);
    # fall back to the split affine*mu form when a_mu2 is too small for the
    # cancellation in k_cs to stay accurate in f32.
    use_quad_act = abs(a_mu2) > 1e-3
    if use_quad_act:
        h_cs = a_mu / (2.0 * a_mu2)
        k_cs = A0 - a_mu2 * h_cs * h_cs
    else:
        h_cs = k_cs = 0.0

    nc = bacc.Bacc("TRN2", target_bir_lowering=False)
    u_d = nc.declare_dram_parameter("u", [N_CYCLES, B_SH], DT, isOutput=False)
    n_d = nc.declare_dram_parameter("noise", [N_CYCLES, B_SH], DT, isOutput=False)
    y_d = nc.declare_dram_parameter("y", [N_CYCLES, B_SH, NCH], DT, isOutput=True)

    u_v = u_d[:].rearrange("t (p f) -> p t f", p=P)
    n_v = n_d[:].rearrange("t (p f) -> p t f", p=P)
    y_v = y_d[:].rearrange("t (p f) j -> p t f j", p=P)

    NBLK = N_CYCLES // K_BLK

    with TileContext(nc) as tc:
        with (
            tc.tile_pool(name="io", bufs=2) as io_pool,
            tc.tile_pool(name="tmp", bufs=3) as tmp_pool,
            tc.tile_pool(name="state", bufs=1) as st_pool,
        ):
            mu_init = st_pool.tile([P, F], DT)
            nc.vector.memset(mu_init[:], mu0)

            biases = st_pool.tile([P, 4], DT)
            nc.vector.memset(biases[:, 0:1], A0)
            nc.vector.memset(biases[:, 1:2], 0.0)
            nc.vector.memset(biases[:, 2:3], h_cs)
            nc.vector.memset(biases[:, 3:4], k_cs)
            A0_ap = biases[:, 0:1]
            zero_ap = biases[:, 1:2]
            h_ap = biases[:, 2:3]
            k_ap = biases[:, 3:4]

            mu_g = [mu_init[:, g * FG:(g + 1) * FG] for g in range(G)]
            st = [None, None]
            tu = [None] * NBLK
            tn = [None] * NBLK
            yt = [None] * NBLK

            def issue_in(blk):
                t0 = blk * K_BLK
                tu[blk] = io_pool.tile([P, K_BLK, F], DT, tag="u", name=f"u{blk}")
                tn[blk] = io_pool.tile([P, K_BLK, F], DT, tag="n", name=f"n{blk}")
                if blk == 0:
                    # split the cold-start load so step 0 can begin after the
                    # first two rows instead of the whole block
                    nc.sync.dma_start(out=tn[0][:, 0:2, :], in_=n_v[:, 0:2, :])
                    nc.sync.dma_start(out=tu[0][:, 0:2, :], in_=u_v[:, 0:2, :])
                    nc.sync.dma_start(out=tn[0][:, 2:K_BLK, :],
                                      in_=n_v[:, 2:K_BLK, :])
                    nc.sync.dma_start(out=tu[0][:, 2:K_BLK, :],
                                      in_=u_v[:, 2:K_BLK, :])
                    return
                nc.sync.dma_start(out=tu[blk][:], in_=u_v[:, t0:t0 + K_BLK, :])
                nc.sync.dma_start(out=tn[blk][:], in_=n_v[:, t0:t0 + K_BLK, :])

            def S1(g, k):
                """mu-only stage: sigmoid-arg, tanh pack, ACT dispatch."""
                blk = k // K_BLK
                ki = k % K_BLK
                lo, hi = g * FG, (g + 1) * FG
                mu = mu_g[g]
                ytile = yt[blk]
                q = tmp_pool.tile([P, FG], DT, tag=f"q{g}", name=f"q{g}_{k}")
                z = tmp_pool.tile([P, 2 * FG], DT, tag=f"z{g}", name=f"z{g}_{k}")
                nc.vector.tensor_scalar(z[:, 0:FG], mu, a1, b1, OP.mult, OP.add)
                nc.vector.tensor_scalar(z[:, FG:2 * FG], mu, a2, b2,
                                        OP.mult, OP.add)
                if use_quad_act:
                    nc.scalar.activation(q[:], mu, AF.Square, bias=h_ap,
                                         scale=1.0)
                else:
                    qv = tmp_pool.tile([P, FG], DT, tag=f"qv{g}",
                                       name=f"qv{g}_{k}")
                    nc.gpsimd.tensor_scalar(qv[:], mu, a_mu2, a_mu,
                                            OP.mult, OP.add)
                    nc.vector.tensor_tensor(q[:], qv[:], mu, OP.mult)
                Tt_tile = tmp_pool.tile([P, 2 * FG], DT, tag=f"T{g}",
                                        name=f"T{g}_{k}")
                Tt = Tt_tile[:]
                nc.scalar.activation(Tt, z[:], AF.Tanh, bias=zero_ap, scale=1.0)
                if use_quad_act:
                    nc.scalar.activation(ytile[:, ki, lo:hi, 2], q[:],
                                         AF.Sigmoid, bias=k_ap, scale=a_mu2)
                else:
                    nc.scalar.activation(ytile[:, ki, lo:hi, 2], q[:],
                                         AF.Sigmoid, bias=A0_ap, scale=1.0)
                st[g] = (q, Tt)

            def S2(g, k):
                """post-ACT stage: cp, sigmas, branches, select, clip."""
                blk = k // K_BLK
                ki = k % K_BLK
                lo, hi = g * FG, (g + 1) * FG
                mu = mu_g[g]
                ytile = yt[blk]
                _, Tt = st[g]
                T1 = Tt[:, 0:FG]
                T2 = Tt[:, FG:2 * FG]
                u_s = tu[blk][:, ki, lo:hi]
                n_s = tn[blk][:, ki, lo:hi]
                o_mu = ytile[:, ki, lo:hi, 0]
                o_cp = ytile[:, ki, lo:hi, 1]
                o_pi = ytile[:, ki, lo:hi, 2]
                o_s1 = ytile[:, ki, lo:hi, 4]
                o_s2 = ytile[:, ki, lo:hi, 6]

                m1 = tmp_pool.tile([P, FG], DT, tag=f"m1{g}", name=f"m1{g}_{k}")
                m2 = tmp_pool.tile([P, FG], DT, tag=f"m2{g}", name=f"m2{g}_{k}")
                e1 = tmp_pool.tile([P, FG], DT, tag=f"e1{g}", name=f"e1{g}_{k}")
                e2 = tmp_pool.tile([P, FG], DT, tag=f"e2{g}", name=f"e2{g}_{k}")
                D1m = tmp_pool.tile([P, FG], DT, tag=f"D1m{g}", name=f"D1m{g}_{k}")
                D2m = tmp_pool.tile([P, FG], DT, tag=f"D2m{g}", name=f"D2m{g}_{k}")

                nc.vector.tensor_tensor(o_cp, u_s, o_pi, OP.is_ge)
                nc.vector.tensor_scalar(o_s1, T1, c21, c01, OP.mult, OP.add)
                nc.gpsimd.tensor_scalar(o_s2, T2, c22, c02, OP.mult, OP.add)
                nc.vector.tensor_tensor(m1[:], o_s1, n_s, OP.mult)
                nc.gpsimd.tensor_tensor(m2[:], o_s2, n_s, OP.mult)
                # pre-add mu into the branch bases; d outputs are batched later
                nc.gpsimd.tensor_scalar(D1m[:], mu, 1.0 + c_mu, D1b,
                                        OP.mult, OP.add)
                nc.gpsimd.tensor_scalar(D2m[:], mu, 1.0 + j_mu, D2b,
                                        OP.mult, OP.add)
                nc.vector.tensor_tensor(e1[:], m1[:], D1m[:], OP.add)
                nc.gpsimd.tensor_tensor(e2[:], m2[:], D2m[:], OP.add)
                nc.vector.copy_predicated(
                    e1[:], o_cp.bitcast(mybir.dt.uint32), e2[:])
                nc.vector.tensor_scalar(o_mu, e1[:], MU_MIN, MU_MAX,
                                        OP.max, OP.min)
                mu_g[g] = o_mu

            def finish_block(blk):
                """batched d1/d2 writeback over stored mu history + out DMA."""
                t0 = blk * K_BLK
                yb = yt[blk]
                if K_BLK > 1:
                    nc.vector.tensor_scalar(yb[:, 1:K_BLK, :, 3],
                                            yb[:, 0:K_BLK - 1, :, 0],
                                            c_mu, D1b, OP.mult, OP.add)
                    nc.vector.tensor_scalar(yb[:, 1:K_BLK, :, 5],
                                            yb[:, 0:K_BLK - 1, :, 0],
                                            j_mu, D2b, OP.mult, OP.add)
                if blk == 0:
                    mu_prev = mu_init[:]
                else:
                    mu_prev = yt[blk - 1][:, K_BLK - 1, :, 0]
                nc.vector.tensor_scalar(yb[:, 0, :, 3], mu_prev,
                                        c_mu, D1b, OP.mult, OP.add)
                nc.vector.tensor_scalar(yb[:, 0, :, 5], mu_prev,
                                        j_mu, D2b, OP.mult, OP.add)
                nc.sync.dma_start(out=y_v[:, t0:t0 + K_BLK, :, :], in_=yb[:])

            issue_in(0)
            for k in range(N_CYCLES):
                blk = k // K_BLK
                if k % K_BLK == 0:
                    yt[blk] = io_pool.tile([P, K_BLK, F, NCH], DT, tag="y",
                                           name=f"y{blk}")
                    if blk + 1 < NBLK:
                        issue_in(blk + 1)
                S1(0, k)
                if k > 0:
                    S2(1, k - 1)
                    if k % K_BLK == 0:
                        finish_block(blk - 1)
                S1(1, k)
                S2(0, k)
            S2(1, N_CYCLES - 1)
            finish_block(NBLK - 1)

    return nc


_CACHE = {}


def _get_nc(consts):
    key = tuple(np.float64(consts).tobytes())
    if key not in _CACHE:
        nc = _build_nc(consts)
        nc.finalize()
        _CACHE[key] = nc
    return _CACHE[key]


def kernel(params, T, u, noise):
    params = np.asarray(params, dtype=np.float32)
    u = np.ascontiguousarray(np.asarray(u, dtype=np.float32))
    noise = np.ascontiguousarray(np.asarray(noise, dtype=np.float32))
    consts = _prep_consts(params, float(np.asarray(T)))
    nc = _get_nc(consts)

    in_maps = []
    for c in range(N_CORES):
        sl = slice(c * B_SH, (c + 1) * B_SH)
        in_maps.append({
            "u": np.ascontiguousarray(u[:, sl]),
            "noise": np.ascontiguousarray(noise[:, sl]),
        })
    res = run_bass_kernel_spmd(nc, in_maps, list(range(N_CORES)))
    out = np.empty((NCH, N_CYCLES, BATCH), np.float32)
    for c in range(N_CORES):
        sl = slice(c * B_SH, (c + 1) * B_SH)
        out[:, :, sl] = res.results[c]["y"].transpose(2, 0, 1)
    return out


if __name__ == "__main__":
    rng = np.random.default_rng(0)
    params = np.array([2.0, -0.1, -1.0, 0.5, 0.01, -0.02, 0.001, -3.0, 1.0, 0.1,
                       0.5, -1.0, 0.02, -1.5, 0.5, 0.12, 0.005], np.float32)
    u = rng.random((N_CYCLES, BATCH), dtype=np.float32)
    noise = rng.standard_normal((N_CYCLES, BATCH), dtype=np.float32)
    y = kernel(params=params, T=np.float32(200.0), u=u, noise=noise)
    print("out", y.shape, y.dtype, float(y[0].mean()))


# revision 9
# speedup vs baseline: 1.0683x; 1.0101x over previous
"""Trainium2 Bass kernel for InteractiveGallingModelV6 batched simulation.

Strategy (tuned via TimelineSim cost-model profiling; ~1.23x the previous
working kernel, 415.8us -> 338.3us simulated per-core):

- Data-parallel over B=65536: 8 cores x 8192 elements, [128 part x 64 free].
- The 150-step recurrence is the whole problem: a single dependency chain is
  latency-bound (~2.7us/step). The batch is split into G=2 independent groups
  of [128 x 32] whose chains the tile scheduler interleaves across engines,
  and each group-step is emitted in two software-pipelined phases
  (S1 = mu-only work + ACT dispatch, S2 = post-ACT work) with half-step skew:
      phase 2k:   S1(g0, k) ; S2(g1, k-1)
      phase 2k+1: S1(g1, k) ; S2(g0, k)
  so every engine FIFO holds ready work while the other group's ACT round
  trip is in flight.
- ACT ops per group-step: Square (completing the square for the sigmoid
  argument), Sigmoid (pi, writes the output slice directly), and ONE wide
  Tanh over a packed [128, 64] tile holding both softplus-fit arguments.
  All three live in the 'sigmoid_and_others' table set (no table switches).
  For |a_mu2| <= 1e-3 the completing-the-square constants blow up, so the
  sigmoid argument falls back to (a_mu2*mu + a_mu)*mu + A0 computed with a
  Pool tensor_scalar + DVE tensor_tensor (both compile-safe op/engine pairs).
- softplus(s0+s_mu*mu+s_T*dT) is approximated as c0 + c2*tanh(a*mu+b) (host
  fit at call time, max fit err ~1e-4; validated end-to-end rel err ~2e-4
  with 1 component flip in 9.8M).
- mu-update pre-adds mu into the branch bases: e_b = s_b*n + ((1+coef)*mu +
  const), so mu' = clip(select(cp, e1, e2)) needs no separate mu+delta add.
  The d1/d2 output channels are then written by per-block BATCHED
  tensor_scalar ops over the stored mu history (15x fixed-cost amortization)
  instead of per-step ops.
- Outputs are staged channel-interleaved [P, K, F, 7] so the output DMA's
  innermost contiguous element is 64*7*4 = 1792B: full DMA rate. (The plain
  per-channel layout's 256B lines run at half rate per the DMA cost model's
  <512B penalty.) The device returns y_dev[t, b, 7]; the host transposes to
  [7, t, b] (pure layout permute).
- Engine assignment tuned empirically (DVE tensor_scalar has a 2x f32 perf
  mode; Pool runs tensor_scalar/tensor_tensor add/mult only -- the backend
  rejects scalar_tensor_tensor and is_ge on Pool).
- Input DMAs for block k+1 are issued before block k's output DMA so the
  in-order SP queue cannot starve the prefetch.
"""
import numpy as np

import concourse.bass as bass
import concourse.bacc as bacc
import concourse.mybir as mybir
from concourse.tile import TileContext
from concourse.bass_utils import run_bass_kernel_spmd

f32 = np.float32
DT = mybir.dt.float32
OP = mybir.AluOpType
AF = mybir.ActivationFunctionType

T_REF = 160.0
MU_MIN, MU_MAX = 0.1, 1.3
N_CYCLES, BATCH = 150, 65536
N_CORES = 8
B_SH = BATCH // N_CORES          # 8192 per core
P = 128
F = B_SH // P                    # 64
G = 2                            # pipelined groups per core
FG = F // G                      # 32
K_BLK = 10                       # steps per DMA block (150 % 10 == 0)
NCH = 7

PARAM_NAMES = ['a0', 'a_T', 'a_mu', 'a_mu2', 'c0', 'c_mu', 'c_T', 's0', 's_mu', 's_T',
               'j0', 'j_mu', 'j_T', 'v0', 'v_mu', 'mu0_base', 'mu0_T']


def _softplus64(x):
    return np.logaddexp(0.0, x)


def _fit_tanh_model(mu_grid, f_vals):
    """Fit f(mu) ~= c0 + c2*tanh(a*mu + b) (coarse-to-fine in (a,b), lstsq
    for the linear coefficients). Returns (a, b, c0, c2)."""
    best = None
    a_grid = np.linspace(0.1, 5.0, 60)
    b_grid = np.linspace(-5.0, 5.0, 101)
    ones = np.ones_like(mu_grid)
    for _ in range(5):
        for a in a_grid:
            for b in b_grid:
                t = np.tanh(a * mu_grid + b)
                A = np.stack([ones, t], 1)
                c, *_ = np.linalg.lstsq(A, f_vals, rcond=None)
                err = np.max(np.abs(A @ c - f_vals))
                if best is None or err < best[0]:
                    best = (err, a, b, c)
        _, a0_, b0_, _ = best
        da = a_grid[1] - a_grid[0]
        db = b_grid[1] - b_grid[0]
        a_grid = np.linspace(a0_ - da, a0_ + da, 21)
        b_grid = np.linspace(b0_ - db, b0_ + db, 21)
    _, a, b, c = best
    return float(a), float(b), float(c[0]), float(c[1])


def _prep_consts(params, T):
    p = {n: float(params[i]) for i, n in enumerate(PARAM_NAMES)}
    dT = float(T) - T_REF
    a_mu2 = p['a_mu2']
    A0 = p['a0'] + p['a_T'] * dT
    mu_grid = np.linspace(MU_MIN, MU_MAX, 4001)
    a1, b1, c01, c21 = _fit_tanh_model(
        mu_grid, _softplus64(p['s0'] + p['s_mu'] * mu_grid + p['s_T'] * dT))
    a2, b2, c02, c22 = _fit_tanh_model(
        mu_grid, _softplus64(p['v0'] + p['v_mu'] * mu_grid))
    D1b = p['c0'] + p['c_T'] * dT
    D2b = p['j0'] + p['j_T'] * dT
    mu0 = float(np.clip(np.float32(p['mu0_base']) + np.float32(p['mu0_T'] * dT),
                        MU_MIN, MU_MAX))
    return (p['a_mu'], a_mu2, A0, a1, b1, c01, c21, a2, b2, c02, c22,
            p['c_mu'], D1b, p['j_mu'], D2b, mu0)


def _build_nc(consts):
    (a_mu, a_mu2, A0, a1, b1, c01, c21, a2, b2, c02, c22,
     c_mu, D1b, j_mu, D2b, mu0) = [float(v) for v in consts]

    # completing-the-square constants for pi = sigmoid(a_mu2*(mu+h)^2 + k);
    # fall back to the split affine*mu form when a_mu2 is too small for the
    # cancellation in k_cs to stay accurate in f32.
    use_quad_act = abs(a_mu2) > 1e-3
    if use_quad_act:
        h_cs = a_mu / (2.0 * a_mu2)
        k_cs = A0 - a_mu2 * h_cs * h_cs
    else:
        h_cs = k_cs = 0.0

    nc = bacc.Bacc("TRN2", target_bir_lowering=False)
    u_d = nc.declare_dram_parameter("u", [N_CYCLES, B_SH], DT, isOutput=False)
    n_d = nc.declare_dram_parameter("noise", [N_CYCLES, B_SH], DT, isOutput=False)
    y_d = nc.declare_dram_parameter("y", [N_CYCLES, B_SH, NCH], DT, isOutput=True)

    u_v = u_d[:].rearrange("t (p f) -> p t f", p=P)
    n_v = n_d[:].rearrange("t (p f) -> p t f", p=P)
    y_v = y_d[:].rearrange("t (p f) j -> p t f j", p=P)

    NBLK = N_CYCLES // K_BLK

    with TileContext(nc) as tc:
        with (
            tc.tile_pool(name="io", bufs=2) as io_pool,
            tc.tile_pool(name="tmp", bufs=3) as tmp_pool,
            tc.tile_pool(name="state", bufs=1) as st_pool,
        ):
            mu_init = st_pool.tile([P, F], DT)
            nc.vector.memset(mu_init[:], mu0)

            biases = st_pool.tile([P, 4], DT)
            nc.vector.memset(biases[:, 0:1], A0)
            nc.vector.memset(biases[:, 1:2], 0.0)
            nc.vector.memset(biases[:, 2:3], h_cs)
            nc.vector.memset(biases[:, 3:4], k_cs)
            A0_ap = biases[:, 0:1]
            zero_ap = biases[:, 1:2]
            h_ap = biases[:, 2:3]
            k_ap = biases[:, 3:4]

            mu_g = [mu_init[:, g * FG:(g + 1) * FG] for g in range(G)]
            st = [None, None]
            tu = [None] * NBLK
            tn = [None] * NBLK
            yt = [None] * NBLK

            def issue_in(blk):
                t0 = blk * K_BLK
                tu[blk] = io_pool.tile([P, K_BLK, F], DT, tag="u", name=f"u{blk}")
                tn[blk] = io_pool.tile([P, K_BLK, F], DT, tag="n", name=f"n{blk}")
                if blk == 0:
                    # split the cold-start load so step 0 can begin after the
                    # first two rows instead of the whole block
                    nc.sync.dma_start(out=tn[0][:, 0:2, :], in_=n_v[:, 0:2, :])
                    nc.sync.dma_start(out=tu[0][:, 0:2, :], in_=u_v[:, 0:2, :])
                    nc.sync.dma_start(out=tn[0][:, 2:K_BLK, :],
                                      in_=n_v[:, 2:K_BLK, :])
                    nc.sync.dma_start(out=tu[0][:, 2:K_BLK, :],
                                      in_=u_v[:, 2:K_BLK, :])
                    return
                nc.sync.dma_start(out=tu[blk][:], in_=u_v[:, t0:t0 + K_BLK, :])
                nc.sync.dma_start(out=tn[blk][:], in_=n_v[:, t0:t0 + K_BLK, :])

            def S1(g, k):
                """mu-only stage: sigmoid-arg, tanh pack, ACT dispatch."""
                blk = k // K_BLK
                ki = k % K_BLK
                lo, hi = g * FG, (g + 1) * FG
                mu = mu_g[g]
                ytile = yt[blk]
                q = tmp_pool.tile([P, FG], DT, tag=f"q{g}", name=f"q{g}_{k}")
                z = tmp_pool.tile([P, 2 * FG], DT, tag=f"z{g}", name=f"z{g}_{k}")
                nc.vector.tensor_scalar(z[:, 0:FG], mu, a1, b1, OP.mult, OP.add)
                nc.vector.tensor_scalar(z[:, FG:2 * FG], mu, a2, b2,
                                        OP.mult, OP.add)
                if use_quad_act:
                    nc.scalar.activation(q[:], mu, AF.Square, bias=h_ap,
                                         scale=1.0)
                else:
                    qv = tmp_pool.tile([P, FG], DT, tag=f"qv{g}",
                                       name=f"qv{g}_{k}")
                    nc.gpsimd.tensor_scalar(qv[:], mu, a_mu2, a_mu,
                                            OP.mult, OP.add)
                    nc.vector.tensor_tensor(q[:], qv[:], mu, OP.mult)
                Tt_tile = tmp_pool.tile([P, 2 * FG], DT, tag=f"T{g}",
                                        name=f"T{g}_{k}")
                Tt = Tt_tile[:]
                nc.scalar.activation(Tt, z[:], AF.Tanh, bias=zero_ap, scale=1.0)
                if use_quad_act:
                    nc.scalar.activation(ytile[:, ki, lo:hi, 2], q[:],
                                         AF.Sigmoid, bias=k_ap, scale=a_mu2)
                else:
                    nc.scalar.activation(ytile[:, ki, lo:hi, 2], q[:],
                                         AF.Sigmoid, bias=A0_ap, scale=1.0)
                st[g] = (q, Tt)

            def S2(g, k):
                """post-ACT stage: cp, sigmas, branches, select, clip."""
                blk = k // K_BLK
                ki = k % K_BLK
                lo, hi = g * FG, (g + 1) * FG
                mu = mu_g[g]
                ytile = yt[blk]
                _, Tt = st[g]
                T1 = Tt[:, 0:FG]
                T2 = Tt[:, FG:2 * FG]
                u_s = tu[blk][:, ki, lo:hi]
                n_s = tn[blk][:, ki, lo:hi]
                o_mu = ytile[:, ki, lo:hi, 0]
                o_cp = ytile[:, ki, lo:hi, 1]
                o_pi = ytile[:, ki, lo:hi, 2]
                o_s1 = ytile[:, ki, lo:hi, 4]
                o_s2 = ytile[:, ki, lo:hi, 6]

                m1 = tmp_pool.tile([P, FG], DT, tag=f"m1{g}", name=f"m1{g}_{k}")
                m2 = tmp_pool.tile([P, FG], DT, tag=f"m2{g}", name=f"m2{g}_{k}")
                e1 = tmp_pool.tile([P, FG], DT, tag=f"e1{g}", name=f"e1{g}_{k}")
                e2 = tmp_pool.tile([P, FG], DT, tag=f"e2{g}", name=f"e2{g}_{k}")
                D1m = tmp_pool.tile([P, FG], DT, tag=f"D1m{g}", name=f"D1m{g}_{k}")
                D2m = tmp_pool.tile([P, FG], DT, tag=f"D2m{g}", name=f"D2m{g}_{k}")

                nc.vector.tensor_tensor(o_cp, u_s, o_pi, OP.is_ge)
                nc.vector.tensor_scalar(o_s1, T1, c21, c01, OP.mult, OP.add)
                nc.gpsimd.tensor_scalar(o_s2, T2, c22, c02, OP.mult, OP.add)
                nc.vector.tensor_tensor(m1[:], o_s1, n_s, OP.mult)
                nc.gpsimd.tensor_tensor(m2[:], o_s2, n_s, OP.mult)
                # pre-add mu into the branch bases; d outputs are batched later
                nc.gpsimd.tensor_scalar(D1m[:], mu, 1.0 + c_mu, D1b,
                                        OP.mult, OP.add)
                nc.gpsimd.tensor_scalar(D2m[:], mu, 1.0 + j_mu, D2b,
                                        OP.mult, OP.add)
                nc.vector.tensor_tensor(e1[:], m1[:], D1m[:], OP.add)
                nc.gpsimd.tensor_tensor(e2[:], m2[:], D2m[:], OP.add)
                nc.vector.copy_predicated(
                    e1[:], o_cp.bitcast(mybir.dt.uint32), e2[:])
                nc.vector.tensor_scalar(o_mu, e1[:], MU_MIN, MU_MAX,
                                        OP.max, OP.min)
                mu_g[g] = o_mu

            def finish_block(blk, r0=0, r1=K_BLK):
                """batched d1/d2 writeback over stored mu history + out DMA
                for rows [r0:r1] of the block."""
                t0 = blk * K_BLK
                yb = yt[blk]
                lo = max(r0, 1)
                if r1 > lo:
                    nc.vector.tensor_scalar(yb[:, lo:r1, :, 3],
                                            yb[:, lo - 1:r1 - 1, :, 0],
                                            c_mu, D1b, OP.mult, OP.add)
                    nc.vector.tensor_scalar(yb[:, lo:r1, :, 5],
                                            yb[:, lo - 1:r1 - 1, :, 0],
                                            j_mu, D2b, OP.mult, OP.add)
                if r0 == 0:
                    if blk == 0:
                        mu_prev = mu_init[:]
                    else:
                        mu_prev = yt[blk - 1][:, K_BLK - 1, :, 0]
                    nc.vector.tensor_scalar(yb[:, 0, :, 3], mu_prev,
                                            c_mu, D1b, OP.mult, OP.add)
                    nc.vector.tensor_scalar(yb[:, 0, :, 5], mu_prev,
                                            j_mu, D2b, OP.mult, OP.add)
                nc.sync.dma_start(out=y_v[:, t0 + r0:t0 + r1, :, :],
                                  in_=yb[:, r0:r1, :, :])

            issue_in(0)
            for k in range(N_CYCLES):
                blk = k // K_BLK
                if k % K_BLK == 0:
                    yt[blk] = io_pool.tile([P, K_BLK, F, NCH], DT, tag="y",
                                           name=f"y{blk}")
                    if blk + 1 < NBLK:
                        issue_in(blk + 1)
                S1(0, k)
                if k > 0:
                    S2(1, k - 1)
                    if k % K_BLK == 0:
                        finish_block(blk - 1)
                    if k == N_CYCLES - K_BLK // 2:
                        # drain the last block's first half early so the final
                        # DMA overlaps the remaining steps' compute
                        finish_block(NBLK - 1, 0, K_BLK // 2)
                S1(1, k)
                S2(0, k)
            S2(1, N_CYCLES - 1)
            finish_block(NBLK - 1, K_BLK // 2, K_BLK)

    return nc


_CACHE = {}


def _get_nc(consts):
    key = tuple(np.float64(consts).tobytes())
    if key not in _CACHE:
        nc = _build_nc(consts)
        nc.finalize()
        _CACHE[key] = nc
    return _CACHE[key]


def kernel(params, T, u, noise):
    params = np.asarray(params, dtype=np.float32)
    u = np.ascontiguousarray(np.asarray(u, dtype=np.float32))
    noise = np.ascontiguousarray(np.asarray(noise, dtype=np.float32))
    consts = _prep_consts(params, float(np.asarray(T)))
    nc = _get_nc(consts)

    in_maps = []
    for c in range(N_CORES):
        sl = slice(c * B_SH, (c + 1) * B_SH)
        in_maps.append({
            "u": np.ascontiguousarray(u[:, sl]),
            "noise": np.ascontiguousarray(noise[:, sl]),
        })
    res = run_bass_kernel_spmd(nc, in_maps, list(range(N_CORES)))
    out = np.empty((NCH, N_CYCLES, BATCH), np.float32)
    for c in range(N_CORES):
        sl = slice(c * B_SH, (c + 1) * B_SH)
        out[:, :, sl] = res.results[c]["y"].transpose(2, 0, 1)
    return out


if __name__ == "__main__":
    rng = np.random.default_rng(0)
    params = np.array([2.0, -0.1, -1.0, 0.5, 0.01, -0.02, 0.001, -3.0, 1.0, 0.1,
                       0.5, -1.0, 0.02, -1.5, 0.5, 0.12, 0.005], np.float32)
    u = rng.random((N_CYCLES, BATCH), dtype=np.float32)
    noise = rng.standard_normal((N_CYCLES, BATCH), dtype=np.float32)
    y = kernel(params=params, T=np.float32(200.0), u=u, noise=noise)
    print("out", y.shape, y.dtype, float(y[0].mean()))


# revision 11
# speedup vs baseline: 1.0742x; 1.0055x over previous
"""Trainium2 Bass kernel for InteractiveGallingModelV6 batched simulation.

Strategy (tuned via TimelineSim cost-model profiling; ~1.24x the previous
working kernel, 415.8us -> 334.9us simulated per-core):

- Data-parallel over B=65536: 8 cores x 8192 elements, [128 part x 64 free].
- The 150-step recurrence is the whole problem: a single dependency chain is
  latency-bound (~2.7us/step). The batch is split into G=2 independent groups
  of [128 x 32] whose chains the tile scheduler interleaves across engines,
  and each group-step is emitted in two software-pipelined phases
  (S1 = mu-only work + ACT dispatch, S2 = post-ACT work) with half-step skew:
      phase 2k:   S1(g0, k) ; S2(g1, k-1)
      phase 2k+1: S1(g1, k) ; S2(g0, k)
  so every engine FIFO holds ready work while the other group's ACT round
  trip is in flight.
- ACT ops per group-step: Square (completing the square for the sigmoid
  argument), Sigmoid (pi, writes the output slice directly), and ONE wide
  Tanh over a packed [128, 64] tile holding both softplus-fit arguments.
  All three live in the 'sigmoid_and_others' table set (no table switches).
  For |a_mu2| <= 1e-3 the completing-the-square constants blow up, so the
  sigmoid argument falls back to (a_mu2*mu + a_mu)*mu + A0 computed with a
  Pool tensor_scalar + DVE tensor_tensor (both compile-safe op/engine pairs).
- softplus(s0+s_mu*mu+s_T*dT) is approximated as c0 + c2*tanh(a*mu+b) (host
  fit at call time, max fit err ~1e-4; validated end-to-end rel err ~2e-4
  with 1 component flip in 9.8M).
- mu-update pre-adds mu into the branch bases: e_b = s_b*n + ((1+coef)*mu +
  const), so mu' = clip(select(cp, e1, e2)) needs no separate mu+delta add.
  The d1/d2 output channels are then written by per-block BATCHED
  tensor_scalar ops over the stored mu history (15x fixed-cost amortization)
  instead of per-step ops.
- Outputs are staged channel-interleaved [P, K, F, 7] so the output DMA's
  innermost contiguous element is 64*7*4 = 1792B: full DMA rate. (The plain
  per-channel layout's 256B lines run at half rate per the DMA cost model's
  <512B penalty.) The device returns y_dev[t, b, 7]; the host transposes to
  [7, t, b] (pure layout permute).
- Engine assignment tuned empirically (DVE tensor_scalar has a 2x f32 perf
  mode; Pool runs tensor_scalar/tensor_tensor add/mult only -- the backend
  rejects scalar_tensor_tensor and is_ge on Pool).
- Input DMAs for block k+1 are issued before block k's output DMA so the
  in-order SP queue cannot starve the prefetch.
"""
import numpy as np

import concourse.bass as bass
import concourse.bacc as bacc
import concourse.mybir as mybir
from concourse.tile import TileContext
from concourse.bass_utils import run_bass_kernel_spmd

f32 = np.float32
DT = mybir.dt.float32
OP = mybir.AluOpType
AF = mybir.ActivationFunctionType

T_REF = 160.0
MU_MIN, MU_MAX = 0.1, 1.3
N_CYCLES, BATCH = 150, 65536
N_CORES = 8
B_SH = BATCH // N_CORES          # 8192 per core
P = 128
F = B_SH // P                    # 64
G = 2                            # pipelined groups per core
FG = F // G                      # 32
K_BLK = 10                       # steps per DMA block (150 % 10 == 0)
NCH = 7

PARAM_NAMES = ['a0', 'a_T', 'a_mu', 'a_mu2', 'c0', 'c_mu', 'c_T', 's0', 's_mu', 's_T',
               'j0', 'j_mu', 'j_T', 'v0', 'v_mu', 'mu0_base', 'mu0_T']


def _softplus64(x):
    return np.logaddexp(0.0, x)


def _fit_tanh_model(mu_grid, f_vals):
    """Fit f(mu) ~= c0 + c2*tanh(a*mu + b) (coarse-to-fine in (a,b), lstsq
    for the linear coefficients). Returns (a, b, c0, c2)."""
    best = None
    a_grid = np.linspace(0.1, 5.0, 60)
    b_grid = np.linspace(-5.0, 5.0, 101)
    ones = np.ones_like(mu_grid)
    for _ in range(5):
        for a in a_grid:
            for b in b_grid:
                t = np.tanh(a * mu_grid + b)
                A = np.stack([ones, t], 1)
                c, *_ = np.linalg.lstsq(A, f_vals, rcond=None)
                err = np.max(np.abs(A @ c - f_vals))
                if best is None or err < best[0]:
                    best = (err, a, b, c)
        _, a0_, b0_, _ = best
        da = a_grid[1] - a_grid[0]
        db = b_grid[1] - b_grid[0]
        a_grid = np.linspace(a0_ - da, a0_ + da, 21)
        b_grid = np.linspace(b0_ - db, b0_ + db, 21)
    _, a, b, c = best
    return float(a), float(b), float(c[0]), float(c[1])


def _prep_consts(params, T):
    p = {n: float(params[i]) for i, n in enumerate(PARAM_NAMES)}
    dT = float(T) - T_REF
    a_mu2 = p['a_mu2']
    A0 = p['a0'] + p['a_T'] * dT
    mu_grid = np.linspace(MU_MIN, MU_MAX, 4001)
    a1, b1, c01, c21 = _fit_tanh_model(
        mu_grid, _softplus64(p['s0'] + p['s_mu'] * mu_grid + p['s_T'] * dT))
    a2, b2, c02, c22 = _fit_tanh_model(
        mu_grid, _softplus64(p['v0'] + p['v_mu'] * mu_grid))
    D1b = p['c0'] + p['c_T'] * dT
    D2b = p['j0'] + p['j_T'] * dT
    mu0 = float(np.clip(np.float32(p['mu0_base']) + np.float32(p['mu0_T'] * dT),
                        MU_MIN, MU_MAX))
    return (p['a_mu'], a_mu2, A0, a1, b1, c01, c21, a2, b2, c02, c22,
            p['c_mu'], D1b, p['j_mu'], D2b, mu0)


def _build_nc(consts):
    (a_mu, a_mu2, A0, a1, b1, c01, c21, a2, b2, c02, c22,
     c_mu, D1b, j_mu, D2b, mu0) = [float(v) for v in consts]

    # completing-the-square constants for pi = sigmoid(a_mu2*(mu+h)^2 + k);
    # fall back to the split affine*mu form when a_mu2 is too small for the
    # cancellation in k_cs to stay accurate in f32.
    use_quad_act = abs(a_mu2) > 1e-3
    if use_quad_act:
        h_cs = a_mu / (2.0 * a_mu2)
        k_cs = A0 - a_mu2 * h_cs * h_cs
    else:
        h_cs = k_cs = 0.0

    nc = bacc.Bacc("TRN2", target_bir_lowering=False)
    u_d = nc.declare_dram_parameter("u", [N_CYCLES, B_SH], DT, isOutput=False)
    n_d = nc.declare_dram_parameter("noise", [N_CYCLES, B_SH], DT, isOutput=False)
    y_d = nc.declare_dram_parameter("y", [N_CYCLES, B_SH, NCH], DT, isOutput=True)

    u_v = u_d[:].rearrange("t (p f) -> p t f", p=P)
    n_v = n_d[:].rearrange("t (p f) -> p t f", p=P)
    y_v = y_d[:].rearrange("t (p f) j -> p t f j", p=P)

    NBLK = N_CYCLES // K_BLK

    with TileContext(nc) as tc:
        with (
            tc.tile_pool(name="io", bufs=2) as io_pool,
            tc.tile_pool(name="tmp", bufs=3) as tmp_pool,
            tc.tile_pool(name="state", bufs=1) as st_pool,
        ):
            mu_init = st_pool.tile([P, F], DT)
            nc.vector.memset(mu_init[:], mu0)

            biases = st_pool.tile([P, 4], DT)
            nc.vector.memset(biases[:, 0:1], A0)
            nc.vector.memset(biases[:, 1:2], 0.0)
            nc.vector.memset(biases[:, 2:3], h_cs)
            nc.vector.memset(biases[:, 3:4], k_cs)
            A0_ap = biases[:, 0:1]
            zero_ap = biases[:, 1:2]
            h_ap = biases[:, 2:3]
            k_ap = biases[:, 3:4]

            mu_g = [mu_init[:, g * FG:(g + 1) * FG] for g in range(G)]
            st = [None, None]
            tu = [None] * NBLK
            tn = [None] * NBLK
            yt = [None] * NBLK

            def issue_in(blk):
                t0 = blk * K_BLK
                tu[blk] = io_pool.tile([P, K_BLK, F], DT, tag="u", name=f"u{blk}")
                tn[blk] = io_pool.tile([P, K_BLK, F], DT, tag="n", name=f"n{blk}")
                if blk == 0:
                    # split the cold-start load so step 0 can begin after the
                    # first two rows instead of the whole block
                    nc.sync.dma_start(out=tn[0][:, 0:2, :], in_=n_v[:, 0:2, :])
                    nc.sync.dma_start(out=tu[0][:, 0:2, :], in_=u_v[:, 0:2, :])
                    nc.sync.dma_start(out=tn[0][:, 2:K_BLK, :],
                                      in_=n_v[:, 2:K_BLK, :])
                    nc.sync.dma_start(out=tu[0][:, 2:K_BLK, :],
                                      in_=u_v[:, 2:K_BLK, :])
                    return
                nc.sync.dma_start(out=tu[blk][:], in_=u_v[:, t0:t0 + K_BLK, :])
                nc.sync.dma_start(out=tn[blk][:], in_=n_v[:, t0:t0 + K_BLK, :])

            def S1(g, k):
                """mu-only stage: sigmoid-arg, tanh pack, ACT dispatch."""
                blk = k // K_BLK
                ki = k % K_BLK
                lo, hi = g * FG, (g + 1) * FG
                mu = mu_g[g]
                ytile = yt[blk]
                q = tmp_pool.tile([P, FG], DT, tag=f"q{g}", name=f"q{g}_{k}")
                z = tmp_pool.tile([P, 2 * FG], DT, tag=f"z{g}", name=f"z{g}_{k}")
                nc.vector.tensor_scalar(z[:, 0:FG], mu, a1, b1, OP.mult, OP.add)
                nc.vector.tensor_scalar(z[:, FG:2 * FG], mu, a2, b2,
                                        OP.mult, OP.add)
                if use_quad_act:
                    nc.scalar.activation(q[:], mu, AF.Square, bias=h_ap,
                                         scale=1.0)
                else:
                    qv = tmp_pool.tile([P, FG], DT, tag=f"qv{g}",
                                       name=f"qv{g}_{k}")
                    nc.gpsimd.tensor_scalar(qv[:], mu, a_mu2, a_mu,
                                            OP.mult, OP.add)
                    nc.vector.tensor_tensor(q[:], qv[:], mu, OP.mult)
                Tt_tile = tmp_pool.tile([P, 2 * FG], DT, tag=f"T{g}",
                                        name=f"T{g}_{k}")
                Tt = Tt_tile[:]
                nc.scalar.activation(Tt, z[:], AF.Tanh, bias=zero_ap, scale=1.0)
                if use_quad_act:
                    nc.scalar.activation(ytile[:, ki, lo:hi, 2], q[:],
                                         AF.Sigmoid, bias=k_ap, scale=a_mu2)
                else:
                    nc.scalar.activation(ytile[:, ki, lo:hi, 2], q[:],
                                         AF.Sigmoid, bias=A0_ap, scale=1.0)
                st[g] = (q, Tt)

            def S2(g, k):
                """post-ACT stage: cp, sigmas, branches, select, clip."""
                blk = k // K_BLK
                ki = k % K_BLK
                lo, hi = g * FG, (g + 1) * FG
                mu = mu_g[g]
                ytile = yt[blk]
                _, Tt = st[g]
                T1 = Tt[:, 0:FG]
                T2 = Tt[:, FG:2 * FG]
                u_s = tu[blk][:, ki, lo:hi]
                n_s = tn[blk][:, ki, lo:hi]
                o_mu = ytile[:, ki, lo:hi, 0]
                o_cp = ytile[:, ki, lo:hi, 1]
                o_pi = ytile[:, ki, lo:hi, 2]
                o_s1 = ytile[:, ki, lo:hi, 4]
                o_s2 = ytile[:, ki, lo:hi, 6]

                m1 = tmp_pool.tile([P, FG], DT, tag=f"m1{g}", name=f"m1{g}_{k}")
                m2 = tmp_pool.tile([P, FG], DT, tag=f"m2{g}", name=f"m2{g}_{k}")
                e1 = tmp_pool.tile([P, FG], DT, tag=f"e1{g}", name=f"e1{g}_{k}")
                e2 = tmp_pool.tile([P, FG], DT, tag=f"e2{g}", name=f"e2{g}_{k}")
                D1m = tmp_pool.tile([P, FG], DT, tag=f"D1m{g}", name=f"D1m{g}_{k}")
                D2m = tmp_pool.tile([P, FG], DT, tag=f"D2m{g}", name=f"D2m{g}_{k}")

                nc.vector.tensor_tensor(o_cp, u_s, o_pi, OP.is_ge)
                nc.vector.tensor_scalar(o_s1, T1, c21, c01, OP.mult, OP.add)
                nc.gpsimd.tensor_scalar(o_s2, T2, c22, c02, OP.mult, OP.add)
                nc.vector.tensor_tensor(m1[:], o_s1, n_s, OP.mult)
                nc.gpsimd.tensor_tensor(m2[:], o_s2, n_s, OP.mult)
                # pre-add mu into the branch bases; d outputs are batched later
                nc.gpsimd.tensor_scalar(D1m[:], mu, 1.0 + c_mu, D1b,
                                        OP.mult, OP.add)
                nc.gpsimd.tensor_scalar(D2m[:], mu, 1.0 + j_mu, D2b,
                                        OP.mult, OP.add)
                nc.vector.tensor_tensor(e1[:], m1[:], D1m[:], OP.add)
                nc.gpsimd.tensor_tensor(e2[:], m2[:], D2m[:], OP.add)
                nc.vector.copy_predicated(
                    e1[:], o_cp.bitcast(mybir.dt.uint32), e2[:])
                nc.vector.tensor_scalar(o_mu, e1[:], MU_MIN, MU_MAX,
                                        OP.max, OP.min)
                mu_g[g] = o_mu

            def finish_block(blk, r0=0, r1=K_BLK):
                """batched d1/d2 writeback over stored mu history + out DMA
                for rows [r0:r1] of the block."""
                t0 = blk * K_BLK
                yb = yt[blk]
                lo = max(r0, 1)
                if r1 > lo:
                    nc.vector.tensor_scalar(yb[:, lo:r1, :, 3],
                                            yb[:, lo - 1:r1 - 1, :, 0],
                                            c_mu, D1b, OP.mult, OP.add)
                    nc.vector.tensor_scalar(yb[:, lo:r1, :, 5],
                                            yb[:, lo - 1:r1 - 1, :, 0],
                                            j_mu, D2b, OP.mult, OP.add)
                if r0 == 0:
                    if blk == 0:
                        mu_prev = mu_init[:]
                    else:
                        mu_prev = yt[blk - 1][:, K_BLK - 1, :, 0]
                    nc.vector.tensor_scalar(yb[:, 0, :, 3], mu_prev,
                                            c_mu, D1b, OP.mult, OP.add)
                    nc.vector.tensor_scalar(yb[:, 0, :, 5], mu_prev,
                                            j_mu, D2b, OP.mult, OP.add)
                nc.sync.dma_start(out=y_v[:, t0 + r0:t0 + r1, :, :],
                                  in_=yb[:, r0:r1, :, :])

            issue_in(0)
            for k in range(N_CYCLES):
                blk = k // K_BLK
                if k % K_BLK == 0:
                    yt[blk] = io_pool.tile([P, K_BLK, F, NCH], DT, tag="y",
                                           name=f"y{blk}")
                    if blk + 1 < NBLK:
                        issue_in(blk + 1)
                S1(0, k)
                if k > 0:
                    S2(1, k - 1)
                    if k % K_BLK == 0:
                        finish_block(blk - 1)
                    if k == N_CYCLES - K_BLK // 2:
                        # drain the last block's first half early so the final
                        # DMA overlaps the remaining steps' compute
                        finish_block(NBLK - 1, 0, K_BLK // 2)
                    if k == N_CYCLES - 2:
                        finish_block(NBLK - 1, K_BLK // 2, K_BLK - 2)
                S1(1, k)
                S2(0, k)
            S2(1, N_CYCLES - 1)
            finish_block(NBLK - 1, K_BLK - 2, K_BLK)

    return nc


_CACHE = {}


def _get_nc(consts):
    key = tuple(np.float64(consts).tobytes())
    if key not in _CACHE:
        nc = _build_nc(consts)
        nc.finalize()
        _CACHE[key] = nc
    return _CACHE[key]


def kernel(params, T, u, noise):
    params = np.asarray(params, dtype=np.float32)
    u = np.ascontiguousarray(np.asarray(u, dtype=np.float32))
    noise = np.ascontiguousarray(np.asarray(noise, dtype=np.float32))
    consts = _prep_consts(params, float(np.asarray(T)))
    nc = _get_nc(consts)

    in_maps = []
    for c in range(N_CORES):
        sl = slice(c * B_SH, (c + 1) * B_SH)
        in_maps.append({
            "u": np.ascontiguousarray(u[:, sl]),
            "noise": np.ascontiguousarray(noise[:, sl]),
        })
    res = run_bass_kernel_spmd(nc, in_maps, list(range(N_CORES)))
    out = np.empty((NCH, N_CYCLES, BATCH), np.float32)
    for c in range(N_CORES):
        sl = slice(c * B_SH, (c + 1) * B_SH)
        out[:, :, sl] = res.results[c]["y"].transpose(2, 0, 1)
    return out


if __name__ == "__main__":
    rng = np.random.default_rng(0)
    params = np.array([2.0, -0.1, -1.0, 0.5, 0.01, -0.02, 0.001, -3.0, 1.0, 0.1,
                       0.5, -1.0, 0.02, -1.5, 0.5, 0.12, 0.005], np.float32)
    u = rng.random((N_CYCLES, BATCH), dtype=np.float32)
    noise = rng.standard_normal((N_CYCLES, BATCH), dtype=np.float32)
    y = kernel(params=params, T=np.float32(200.0), u=u, noise=noise)
    print("out", y.shape, y.dtype, float(y[0].mean()))


# revision 14
# speedup vs baseline: 1.0762x; 1.0018x over previous
"""Trainium2 Bass kernel for InteractiveGallingModelV6 batched simulation.

Strategy (tuned via TimelineSim cost-model profiling; ~1.25x the previous
working kernel, 415.8us -> 333.1us simulated per-core):

- Data-parallel over B=65536: 8 cores x 8192 elements, [128 part x 64 free].
- The 150-step recurrence is the whole problem: a single dependency chain is
  latency-bound (~2.7us/step). The batch is split into G=2 independent groups
  of [128 x 32] whose chains the tile scheduler interleaves across engines,
  and each group-step is emitted in two software-pipelined phases
  (S1 = mu-only work + ACT dispatch, S2 = post-ACT work) with half-step skew:
      phase 2k:   S1(g0, k) ; S2(g1, k-1)
      phase 2k+1: S1(g1, k) ; S2(g0, k)
  so every engine FIFO holds ready work while the other group's ACT round
  trip is in flight.
- ACT ops per group-step: Square (completing the square for the sigmoid
  argument), Sigmoid (pi, writes the output slice directly), and ONE wide
  Tanh over a packed [128, 64] tile holding both softplus-fit arguments.
  All three live in the 'sigmoid_and_others' table set (no table switches).
  For |a_mu2| <= 1e-3 the completing-the-square constants blow up, so the
  sigmoid argument falls back to (a_mu2*mu + a_mu)*mu + A0 computed with a
  Pool tensor_scalar + DVE tensor_tensor (both compile-safe op/engine pairs).
- softplus(s0+s_mu*mu+s_T*dT) is approximated as c0 + c2*tanh(a*mu+b) (host
  fit at call time, max fit err ~1e-4; validated end-to-end rel err ~2e-4
  with 1 component flip in 9.8M).
- mu-update pre-adds mu into the branch bases: e_b = s_b*n + ((1+coef)*mu +
  const), so mu' = clip(select(cp, e1, e2)) needs no separate mu+delta add.
  The d1/d2 output channels are then written by per-block BATCHED
  tensor_scalar ops over the stored mu history (15x fixed-cost amortization)
  instead of per-step ops.
- Outputs are staged channel-interleaved [P, K, F, 7] so the output DMA's
  innermost contiguous element is 64*7*4 = 1792B: full DMA rate. (The plain
  per-channel layout's 256B lines run at half rate per the DMA cost model's
  <512B penalty.) The device returns y_dev[t, b, 7]; the host transposes to
  [7, t, b] (pure layout permute).
- Engine assignment tuned empirically (DVE tensor_scalar has a 2x f32 perf
  mode; Pool runs tensor_scalar/tensor_tensor add/mult only -- the backend
  rejects scalar_tensor_tensor and is_ge on Pool).
- Input DMAs for block k+1 are issued before block k's output DMA so the
  in-order SP queue cannot starve the prefetch.
"""
import numpy as np

import concourse.bass as bass
import concourse.bacc as bacc
import concourse.mybir as mybir
from concourse.tile import TileContext
from concourse.bass_utils import run_bass_kernel_spmd

f32 = np.float32
DT = mybir.dt.float32
OP = mybir.AluOpType
AF = mybir.ActivationFunctionType

T_REF = 160.0
MU_MIN, MU_MAX = 0.1, 1.3
N_CYCLES, BATCH = 150, 65536
N_CORES = 8
B_SH = BATCH // N_CORES          # 8192 per core
P = 128
F = B_SH // P                    # 64
G = 2                            # pipelined groups per core
FG = F // G                      # 32
K_BLK = 10                       # steps per DMA block (150 % 10 == 0)
NCH = 7

PARAM_NAMES = ['a0', 'a_T', 'a_mu', 'a_mu2', 'c0', 'c_mu', 'c_T', 's0', 's_mu', 's_T',
               'j0', 'j_mu', 'j_T', 'v0', 'v_mu', 'mu0_base', 'mu0_T']


def _softplus64(x):
    return np.logaddexp(0.0, x)


def _fit_tanh_model(mu_grid, f_vals):
    """Fit f(mu) ~= c0 + c2*tanh(a*mu + b) (coarse-to-fine in (a,b), lstsq
    for the linear coefficients). Returns (a, b, c0, c2)."""
    best = None
    a_grid = np.linspace(0.1, 5.0, 60)
    b_grid = np.linspace(-5.0, 5.0, 101)
    ones = np.ones_like(mu_grid)
    for _ in range(5):
        for a in a_grid:
            for b in b_grid:
                t = np.tanh(a * mu_grid + b)
                A = np.stack([ones, t], 1)
                c, *_ = np.linalg.lstsq(A, f_vals, rcond=None)
                err = np.max(np.abs(A @ c - f_vals))
                if best is None or err < best[0]:
                    best = (err, a, b, c)
        _, a0_, b0_, _ = best
        da = a_grid[1] - a_grid[0]
        db = b_grid[1] - b_grid[0]
        a_grid = np.linspace(a0_ - da, a0_ + da, 21)
        b_grid = np.linspace(b0_ - db, b0_ + db, 21)
    _, a, b, c = best
    return float(a), float(b), float(c[0]), float(c[1])


def _prep_consts(params, T):
    p = {n: float(params[i]) for i, n in enumerate(PARAM_NAMES)}
    dT = float(T) - T_REF
    a_mu2 = p['a_mu2']
    A0 = p['a0'] + p['a_T'] * dT
    mu_grid = np.linspace(MU_MIN, MU_MAX, 4001)
    a1, b1, c01, c21 = _fit_tanh_model(
        mu_grid, _softplus64(p['s0'] + p['s_mu'] * mu_grid + p['s_T'] * dT))
    a2, b2, c02, c22 = _fit_tanh_model(
        mu_grid, _softplus64(p['v0'] + p['v_mu'] * mu_grid))
    D1b = p['c0'] + p['c_T'] * dT
    D2b = p['j0'] + p['j_T'] * dT
    mu0 = float(np.clip(np.float32(p['mu0_base']) + np.float32(p['mu0_T'] * dT),
                        MU_MIN, MU_MAX))
    return (p['a_mu'], a_mu2, A0, a1, b1, c01, c21, a2, b2, c02, c22,
            p['c_mu'], D1b, p['j_mu'], D2b, mu0)


def _build_nc(consts):
    (a_mu, a_mu2, A0, a1, b1, c01, c21, a2, b2, c02, c22,
     c_mu, D1b, j_mu, D2b, mu0) = [float(v) for v in consts]

    # completing-the-square constants for pi = sigmoid(a_mu2*(mu+h)^2 + k);
    # fall back to the split affine*mu form when a_mu2 is too small for the
    # cancellation in k_cs to stay accurate in f32.
    use_quad_act = abs(a_mu2) > 1e-3
    if use_quad_act:
        h_cs = a_mu / (2.0 * a_mu2)
        k_cs = A0 - a_mu2 * h_cs * h_cs
    else:
        h_cs = k_cs = 0.0

    nc = bacc.Bacc("TRN2", target_bir_lowering=False)
    u_d = nc.declare_dram_parameter("u", [N_CYCLES, B_SH], DT, isOutput=False)
    n_d = nc.declare_dram_parameter("noise", [N_CYCLES, B_SH], DT, isOutput=False)
    y_d = nc.declare_dram_parameter("y", [N_CYCLES, B_SH, NCH], DT, isOutput=True)

    u_v = u_d[:].rearrange("t (p f) -> p t f", p=P)
    n_v = n_d[:].rearrange("t (p f) -> p t f", p=P)
    y_v = y_d[:].rearrange("t (p f) j -> p t f j", p=P)

    NBLK = N_CYCLES // K_BLK

    with TileContext(nc) as tc:
        with (
            tc.tile_pool(name="io", bufs=2) as io_pool,
            tc.tile_pool(name="tmp", bufs=3) as tmp_pool,
            tc.tile_pool(name="state", bufs=1) as st_pool,
        ):
            mu_init = st_pool.tile([P, F], DT)
            nc.vector.memset(mu_init[:], mu0)

            biases = st_pool.tile([P, 4], DT)
            nc.vector.memset(biases[:, 0:1], A0)
            nc.vector.memset(biases[:, 1:2], 0.0)
            nc.vector.memset(biases[:, 2:3], h_cs)
            nc.vector.memset(biases[:, 3:4], k_cs)
            A0_ap = biases[:, 0:1]
            zero_ap = biases[:, 1:2]
            h_ap = biases[:, 2:3]
            k_ap = biases[:, 3:4]

            mu_g = [mu_init[:, g * FG:(g + 1) * FG] for g in range(G)]
            st = [None, None]
            tu = [None] * NBLK
            tn = [None] * NBLK
            yt = [None] * NBLK

            def issue_in(blk):
                t0 = blk * K_BLK
                tu[blk] = io_pool.tile([P, K_BLK, F], DT, tag="u", name=f"u{blk}")
                tn[blk] = io_pool.tile([P, K_BLK, F], DT, tag="n", name=f"n{blk}")
                if blk == 0:
                    # split the cold-start load so step 0 can begin after the
                    # first two rows instead of the whole block
                    nc.sync.dma_start(out=tn[0][:, 0:2, :], in_=n_v[:, 0:2, :])
                    nc.sync.dma_start(out=tu[0][:, 0:2, :], in_=u_v[:, 0:2, :])
                    nc.sync.dma_start(out=tn[0][:, 2:K_BLK, :],
                                      in_=n_v[:, 2:K_BLK, :])
                    nc.sync.dma_start(out=tu[0][:, 2:K_BLK, :],
                                      in_=u_v[:, 2:K_BLK, :])
                    return
                nc.sync.dma_start(out=tu[blk][:], in_=u_v[:, t0:t0 + K_BLK, :])
                nc.sync.dma_start(out=tn[blk][:], in_=n_v[:, t0:t0 + K_BLK, :])

            def S1(g, k):
                """mu-only stage: sigmoid-arg, tanh pack, ACT dispatch."""
                blk = k // K_BLK
                ki = k % K_BLK
                lo, hi = g * FG, (g + 1) * FG
                mu = mu_g[g]
                ytile = yt[blk]
                q = tmp_pool.tile([P, FG], DT, tag=f"q{g}", name=f"q{g}_{k}")
                z = tmp_pool.tile([P, 2 * FG], DT, tag=f"z{g}", name=f"z{g}_{k}")
                nc.vector.tensor_scalar(z[:, 0:FG], mu, a1, b1, OP.mult, OP.add)
                nc.vector.tensor_scalar(z[:, FG:2 * FG], mu, a2, b2,
                                        OP.mult, OP.add)
                if use_quad_act:
                    nc.scalar.activation(q[:], mu, AF.Square, bias=h_ap,
                                         scale=1.0)
                else:
                    qv = tmp_pool.tile([P, FG], DT, tag=f"qv{g}",
                                       name=f"qv{g}_{k}")
                    nc.gpsimd.tensor_scalar(qv[:], mu, a_mu2, a_mu,
                                            OP.mult, OP.add)
                    nc.vector.tensor_tensor(q[:], qv[:], mu, OP.mult)
                Tt_tile = tmp_pool.tile([P, 2 * FG], DT, tag=f"T{g}",
                                        name=f"T{g}_{k}")
                Tt = Tt_tile[:]
                nc.scalar.activation(Tt, z[:], AF.Tanh, bias=zero_ap, scale=1.0)
                if use_quad_act:
                    nc.scalar.activation(ytile[:, ki, lo:hi, 2], q[:],
                                         AF.Sigmoid, bias=k_ap, scale=a_mu2)
                else:
                    nc.scalar.activation(ytile[:, ki, lo:hi, 2], q[:],
                                         AF.Sigmoid, bias=A0_ap, scale=1.0)
                st[g] = (q, Tt)

            def S2(g, k):
                """post-ACT stage: cp, sigmas, branches, select, clip."""
                blk = k // K_BLK
                ki = k % K_BLK
                lo, hi = g * FG, (g + 1) * FG
                mu = mu_g[g]
                ytile = yt[blk]
                _, Tt = st[g]
                T1 = Tt[:, 0:FG]
                T2 = Tt[:, FG:2 * FG]
                u_s = tu[blk][:, ki, lo:hi]
                n_s = tn[blk][:, ki, lo:hi]
                o_mu = ytile[:, ki, lo:hi, 0]
                o_cp = ytile[:, ki, lo:hi, 1]
                o_pi = ytile[:, ki, lo:hi, 2]
                o_s1 = ytile[:, ki, lo:hi, 4]
                o_s2 = ytile[:, ki, lo:hi, 6]

                m1 = tmp_pool.tile([P, FG], DT, tag=f"m1{g}", name=f"m1{g}_{k}")
                m2 = tmp_pool.tile([P, FG], DT, tag=f"m2{g}", name=f"m2{g}_{k}")
                e1 = tmp_pool.tile([P, FG], DT, tag=f"e1{g}", name=f"e1{g}_{k}")
                e2 = tmp_pool.tile([P, FG], DT, tag=f"e2{g}", name=f"e2{g}_{k}")
                D1m = tmp_pool.tile([P, FG], DT, tag=f"D1m{g}", name=f"D1m{g}_{k}")
                D2m = tmp_pool.tile([P, FG], DT, tag=f"D2m{g}", name=f"D2m{g}_{k}")

                nc.vector.tensor_tensor(o_cp, u_s, o_pi, OP.is_ge)
                nc.vector.tensor_scalar(o_s1, T1, c21, c01, OP.mult, OP.add)
                nc.gpsimd.tensor_scalar(o_s2, T2, c22, c02, OP.mult, OP.add)
                nc.vector.tensor_tensor(m1[:], o_s1, n_s, OP.mult)
                nc.gpsimd.tensor_tensor(m2[:], o_s2, n_s, OP.mult)
                # pre-add mu into the branch bases; d outputs are batched later
                nc.gpsimd.tensor_scalar(D1m[:], mu, 1.0 + c_mu, D1b,
                                        OP.mult, OP.add)
                nc.gpsimd.tensor_scalar(D2m[:], mu, 1.0 + j_mu, D2b,
                                        OP.mult, OP.add)
                nc.vector.tensor_tensor(e1[:], m1[:], D1m[:], OP.add)
                nc.gpsimd.tensor_tensor(e2[:], m2[:], D2m[:], OP.add)
                nc.vector.copy_predicated(
                    e1[:], o_cp.bitcast(mybir.dt.uint32), e2[:])
                nc.vector.tensor_scalar(o_mu, e1[:], MU_MIN, MU_MAX,
                                        OP.max, OP.min)
                mu_g[g] = o_mu

            def finish_block(blk, r0=0, r1=K_BLK):
                """batched d1/d2 writeback over stored mu history + out DMA
                for rows [r0:r1] of the block."""
                t0 = blk * K_BLK
                yb = yt[blk]
                lo = max(r0, 1)
                if r1 > lo:
                    nc.vector.tensor_scalar(yb[:, lo:r1, :, 3],
                                            yb[:, lo - 1:r1 - 1, :, 0],
                                            c_mu, D1b, OP.mult, OP.add)
                    nc.vector.tensor_scalar(yb[:, lo:r1, :, 5],
                                            yb[:, lo - 1:r1 - 1, :, 0],
                                            j_mu, D2b, OP.mult, OP.add)
                if r0 == 0:
                    if blk == 0:
                        mu_prev = mu_init[:]
                    else:
                        mu_prev = yt[blk - 1][:, K_BLK - 1, :, 0]
                    nc.vector.tensor_scalar(yb[:, 0, :, 3], mu_prev,
                                            c_mu, D1b, OP.mult, OP.add)
                    nc.vector.tensor_scalar(yb[:, 0, :, 5], mu_prev,
                                            j_mu, D2b, OP.mult, OP.add)
                nc.sync.dma_start(out=y_v[:, t0 + r0:t0 + r1, :, :],
                                  in_=yb[:, r0:r1, :, :])

            issue_in(0)
            for k in range(N_CYCLES):
                blk = k // K_BLK
                if k % K_BLK == 0:
                    yt[blk] = io_pool.tile([P, K_BLK, F, NCH], DT, tag="y",
                                           name=f"y{blk}")
                    if blk + 1 < NBLK:
                        issue_in(blk + 1)
                S1(0, k)
                if k > 0:
                    S2(1, k - 1)
                    if k % K_BLK == 0:
                        finish_block(blk - 1)
                    if k == N_CYCLES - K_BLK // 2:
                        # drain the last block's first half early so the final
                        # DMA overlaps the remaining steps' compute
                        finish_block(NBLK - 1, 0, K_BLK // 2)
                    if k == N_CYCLES - 2:
                        finish_block(NBLK - 1, K_BLK // 2, K_BLK - 2)
                    if k == N_CYCLES - 1:
                        finish_block(NBLK - 1, K_BLK - 2, K_BLK - 1)
                S1(1, k)
                S2(0, k)
            S2(1, N_CYCLES - 1)
            finish_block(NBLK - 1, K_BLK - 1, K_BLK)

    return nc


_CACHE = {}


def _get_nc(consts):
    key = tuple(np.float64(consts).tobytes())
    if key not in _CACHE:
        nc = _build_nc(consts)
        nc.finalize()
        _CACHE[key] = nc
    return _CACHE[key]


def kernel(params, T, u, noise):
    params = np.asarray(params, dtype=np.float32)
    u = np.ascontiguousarray(np.asarray(u, dtype=np.float32))
    noise = np.ascontiguousarray(np.asarray(noise, dtype=np.float32))
    consts = _prep_consts(params, float(np.asarray(T)))
    nc = _get_nc(consts)

    in_maps = []
    for c in range(N_CORES):
        sl = slice(c * B_SH, (c + 1) * B_SH)
        in_maps.append({
            "u": np.ascontiguousarray(u[:, sl]),
            "noise": np.ascontiguousarray(noise[:, sl]),
        })
    res = run_bass_kernel_spmd(nc, in_maps, list(range(N_CORES)))
    out = np.empty((NCH, N_CYCLES, BATCH), np.float32)
    for c in range(N_CORES):
        sl = slice(c * B_SH, (c + 1) * B_SH)
        out[:, :, sl] = res.results[c]["y"].transpose(2, 0, 1)
    return out


if __name__ == "__main__":
    rng = np.random.default_rng(0)
    params = np.array([2.0, -0.1, -1.0, 0.5, 0.01, -0.02, 0.001, -3.0, 1.0, 0.1,
                       0.5, -1.0, 0.02, -1.5, 0.5, 0.12, 0.005], np.float32)
    u = rng.random((N_CYCLES, BATCH), dtype=np.float32)
    noise = rng.standard_normal((N_CYCLES, BATCH), dtype=np.float32)
    y = kernel(params=params, T=np.float32(200.0), u=u, noise=noise)
    print("out", y.shape, y.dtype, float(y[0].mean()))


# revision 16
# speedup vs baseline: 1.0966x; 1.0190x over previous
"""Trainium2 Bass kernel for InteractiveGallingModelV6 batched simulation.

Strategy (tuned via TimelineSim cost-model profiling; ~1.25x the previous
working kernel, 415.8us -> 332.5us simulated per-core):

- Data-parallel over B=65536: 8 cores x 8192 elements, [128 part x 64 free].
- The 150-step recurrence is the whole problem: a single dependency chain is
  latency-bound (~2.7us/step). The batch is split into G=2 independent groups
  of [128 x 32] whose chains the tile scheduler interleaves across engines,
  and each group-step is emitted in two software-pipelined phases
  (S1 = mu-only work + ACT dispatch, S2 = post-ACT work) with half-step skew:
      phase 2k:   S1(g0, k) ; S2(g1, k-1)
      phase 2k+1: S1(g1, k) ; S2(g0, k)
  so every engine FIFO holds ready work while the other group's ACT round
  trip is in flight.
- ACT ops per group-step: Square (completing the square for the sigmoid
  argument), Sigmoid (pi, writes the output slice directly), and ONE wide
  Tanh over a packed [128, 64] tile holding both softplus-fit arguments.
  All three live in the 'sigmoid_and_others' table set (no table switches).
  For |a_mu2| <= 1e-3 the completing-the-square constants blow up, so the
  sigmoid argument falls back to (a_mu2*mu + a_mu)*mu + A0 computed with a
  Pool tensor_scalar + DVE tensor_tensor (both compile-safe op/engine pairs).
- softplus(s0+s_mu*mu+s_T*dT) is approximated as c0 + c2*tanh(a*mu+b) (host
  fit at call time, max fit err ~1e-4; validated end-to-end rel err ~2e-4
  with 1 component flip in 9.8M).
- mu-update pre-adds mu into the branch bases: e_b = s_b*n + ((1+coef)*mu +
  const), so mu' = clip(select(cp, e1, e2)) needs no separate mu+delta add.
  The d1/d2 output channels are then written by per-block BATCHED
  tensor_scalar ops over the stored mu history (15x fixed-cost amortization)
  instead of per-step ops.
- Outputs are staged channel-interleaved [P, K, F, 7] so the output DMA's
  innermost contiguous element is 64*7*4 = 1792B: full DMA rate. (The plain
  per-channel layout's 256B lines run at half rate per the DMA cost model's
  <512B penalty.) The device returns y_dev[t, b, 7]; the host transposes to
  [7, t, b] (pure layout permute).
- Engine assignment tuned empirically (DVE tensor_scalar has a 2x f32 perf
  mode; Pool runs tensor_scalar/tensor_tensor add/mult only -- the backend
  rejects scalar_tensor_tensor and is_ge on Pool).
- Input DMAs for block k+1 are issued before block k's output DMA so the
  in-order SP queue cannot starve the prefetch.
"""
import numpy as np

import concourse.bass as bass
import concourse.bacc as bacc
import concourse.mybir as mybir
from concourse.tile import TileContext
from concourse.bass_utils import run_bass_kernel_spmd

f32 = np.float32
DT = mybir.dt.float32
OP = mybir.AluOpType
AF = mybir.ActivationFunctionType

T_REF = 160.0
MU_MIN, MU_MAX = 0.1, 1.3
N_CYCLES, BATCH = 150, 65536
N_CORES = 8
B_SH = BATCH // N_CORES          # 8192 per core
P = 128
F = B_SH // P                    # 64
G = 2                            # pipelined groups per core
FG = F // G                      # 32
K_BLK = 10                       # steps per DMA block (150 % 10 == 0)
NCH = 7

PARAM_NAMES = ['a0', 'a_T', 'a_mu', 'a_mu2', 'c0', 'c_mu', 'c_T', 's0', 's_mu', 's_T',
               'j0', 'j_mu', 'j_T', 'v0', 'v_mu', 'mu0_base', 'mu0_T']


def _softplus64(x):
    return np.logaddexp(0.0, x)


def _fit_tanh_model(mu_grid, f_vals):
    """Fit f(mu) ~= c0 + c2*tanh(a*mu + b) (coarse-to-fine in (a,b), lstsq
    for the linear coefficients). Returns (a, b, c0, c2)."""
    best = None
    a_grid = np.linspace(0.1, 5.0, 60)
    b_grid = np.linspace(-5.0, 5.0, 101)
    ones = np.ones_like(mu_grid)
    for _ in range(5):
        for a in a_grid:
            for b in b_grid:
                t = np.tanh(a * mu_grid + b)
                A = np.stack([ones, t], 1)
                c, *_ = np.linalg.lstsq(A, f_vals, rcond=None)
                err = np.max(np.abs(A @ c - f_vals))
                if best is None or err < best[0]:
                    best = (err, a, b, c)
        _, a0_, b0_, _ = best
        da = a_grid[1] - a_grid[0]
        db = b_grid[1] - b_grid[0]
        a_grid = np.linspace(a0_ - da, a0_ + da, 21)
        b_grid = np.linspace(b0_ - db, b0_ + db, 21)
    _, a, b, c = best
    return float(a), float(b), float(c[0]), float(c[1])


def _prep_consts(params, T):
    p = {n: float(params[i]) for i, n in enumerate(PARAM_NAMES)}
    dT = float(T) - T_REF
    a_mu2 = p['a_mu2']
    A0 = p['a0'] + p['a_T'] * dT
    mu_grid = np.linspace(MU_MIN, MU_MAX, 4001)
    a1, b1, c01, c21 = _fit_tanh_model(
        mu_grid, _softplus64(p['s0'] + p['s_mu'] * mu_grid + p['s_T'] * dT))
    a2, b2, c02, c22 = _fit_tanh_model(
        mu_grid, _softplus64(p['v0'] + p['v_mu'] * mu_grid))
    D1b = p['c0'] + p['c_T'] * dT
    D2b = p['j0'] + p['j_T'] * dT
    mu0 = float(np.clip(np.float32(p['mu0_base']) + np.float32(p['mu0_T'] * dT),
                        MU_MIN, MU_MAX))
    return (p['a_mu'], a_mu2, A0, a1, b1, c01, c21, a2, b2, c02, c22,
            p['c_mu'], D1b, p['j_mu'], D2b, mu0)


def _build_nc(consts):
    (a_mu, a_mu2, A0, a1, b1, c01, c21, a2, b2, c02, c22,
     c_mu, D1b, j_mu, D2b, mu0) = [float(v) for v in consts]

    # completing-the-square constants for pi = sigmoid(a_mu2*(mu+h)^2 + k);
    # fall back to the split affine*mu form when a_mu2 is too small for the
    # cancellation in k_cs to stay accurate in f32.
    use_quad_act = abs(a_mu2) > 1e-3
    if use_quad_act:
        h_cs = a_mu / (2.0 * a_mu2)
        k_cs = A0 - a_mu2 * h_cs * h_cs
    else:
        h_cs = k_cs = 0.0

    nc = bacc.Bacc("TRN2", target_bir_lowering=False)
    u_d = nc.declare_dram_parameter("u", [N_CYCLES, B_SH], DT, isOutput=False)
    n_d = nc.declare_dram_parameter("noise", [N_CYCLES, B_SH], DT, isOutput=False)
    y_d = nc.declare_dram_parameter("y", [N_CYCLES, B_SH, NCH], DT, isOutput=True)

    u_v = u_d[:].rearrange("t (p f) -> p t f", p=P)
    n_v = n_d[:].rearrange("t (p f) -> p t f", p=P)
    y_v = y_d[:].rearrange("t (p f) j -> p t f j", p=P)

    NBLK = N_CYCLES // K_BLK

    with TileContext(nc) as tc:
        with (
            tc.tile_pool(name="io", bufs=2) as io_pool,
            tc.tile_pool(name="tmp", bufs=3) as tmp_pool,
            tc.tile_pool(name="state", bufs=1) as st_pool,
        ):
            mu_init = st_pool.tile([P, F], DT)
            nc.vector.memset(mu_init[:], mu0)

            biases = st_pool.tile([P, 4], DT)
            nc.vector.memset(biases[:, 0:1], A0)
            nc.vector.memset(biases[:, 1:2], 0.0)
            nc.vector.memset(biases[:, 2:3], h_cs)
            nc.vector.memset(biases[:, 3:4], k_cs)
            A0_ap = biases[:, 0:1]
            zero_ap = biases[:, 1:2]
            h_ap = biases[:, 2:3]
            k_ap = biases[:, 3:4]

            mu_g = [mu_init[:, g * FG:(g + 1) * FG] for g in range(G)]
            st = [None, None]
            tu = [None] * NBLK
            tn = [None] * NBLK
            yt = [None] * NBLK

            def issue_in(blk):
                t0 = blk * K_BLK
                tu[blk] = io_pool.tile([P, K_BLK, F], DT, tag="u", name=f"u{blk}")
                tn[blk] = io_pool.tile([P, K_BLK, F], DT, tag="n", name=f"n{blk}")
                if blk == 0:
                    # split the cold-start load so step 0 can begin after the
                    # first two rows instead of the whole block
                    nc.sync.dma_start(out=tn[0][:, 0:2, :], in_=n_v[:, 0:2, :])
                    nc.sync.dma_start(out=tu[0][:, 0:2, :], in_=u_v[:, 0:2, :])
                    nc.sync.dma_start(out=tn[0][:, 2:K_BLK, :],
                                      in_=n_v[:, 2:K_BLK, :])
                    nc.sync.dma_start(out=tu[0][:, 2:K_BLK, :],
                                      in_=u_v[:, 2:K_BLK, :])
                    return
                nc.sync.dma_start(out=tu[blk][:], in_=u_v[:, t0:t0 + K_BLK, :])
                nc.sync.dma_start(out=tn[blk][:], in_=n_v[:, t0:t0 + K_BLK, :])

            def S1(g, k):
                """mu-only stage: sigmoid-arg, tanh pack, ACT dispatch."""
                blk = k // K_BLK
                ki = k % K_BLK
                lo, hi = g * FG, (g + 1) * FG
                mu = mu_g[g]
                ytile = yt[blk]
                q = tmp_pool.tile([P, FG], DT, tag=f"q{g}", name=f"q{g}_{k}")
                z = tmp_pool.tile([P, 2 * FG], DT, tag=f"z{g}", name=f"z{g}_{k}")
                nc.vector.tensor_scalar(z[:, 0:FG], mu, a1, b1, OP.mult, OP.add)
                nc.vector.tensor_scalar(z[:, FG:2 * FG], mu, a2, b2,
                                        OP.mult, OP.add)
                if use_quad_act:
                    nc.scalar.activation(q[:], mu, AF.Square, bias=h_ap,
                                         scale=1.0)
                else:
                    qv = tmp_pool.tile([P, FG], DT, tag=f"qv{g}",
                                       name=f"qv{g}_{k}")
                    nc.gpsimd.tensor_scalar(qv[:], mu, a_mu2, a_mu,
                                            OP.mult, OP.add)
                    nc.vector.tensor_tensor(q[:], qv[:], mu, OP.mult)
                Tt_tile = tmp_pool.tile([P, 2 * FG], DT, tag=f"T{g}",
                                        name=f"T{g}_{k}")
                Tt = Tt_tile[:]
                nc.scalar.activation(Tt, z[:], AF.Tanh, bias=zero_ap, scale=1.0)
                if use_quad_act:
                    nc.scalar.activation(ytile[:, ki, lo:hi, 2], q[:],
                                         AF.Sigmoid, bias=k_ap, scale=a_mu2)
                else:
                    nc.scalar.activation(ytile[:, ki, lo:hi, 2], q[:],
                                         AF.Sigmoid, bias=A0_ap, scale=1.0)
                st[g] = (q, Tt)

            def S2(g, k):
                """post-ACT stage: cp, sigmas, branches, select, clip."""
                blk = k // K_BLK
                ki = k % K_BLK
                lo, hi = g * FG, (g + 1) * FG
                mu = mu_g[g]
                ytile = yt[blk]
                _, Tt = st[g]
                T1 = Tt[:, 0:FG]
                T2 = Tt[:, FG:2 * FG]
                u_s = tu[blk][:, ki, lo:hi]
                n_s = tn[blk][:, ki, lo:hi]
                o_mu = ytile[:, ki, lo:hi, 0]
                o_cp = ytile[:, ki, lo:hi, 1]
                o_pi = ytile[:, ki, lo:hi, 2]
                o_s1 = ytile[:, ki, lo:hi, 4]
                o_s2 = ytile[:, ki, lo:hi, 6]

                m1 = tmp_pool.tile([P, FG], DT, tag=f"m1{g}", name=f"m1{g}_{k}")
                m2 = tmp_pool.tile([P, FG], DT, tag=f"m2{g}", name=f"m2{g}_{k}")
                e1 = tmp_pool.tile([P, FG], DT, tag=f"e1{g}", name=f"e1{g}_{k}")
                e2 = tmp_pool.tile([P, FG], DT, tag=f"e2{g}", name=f"e2{g}_{k}")
                D1m = tmp_pool.tile([P, FG], DT, tag=f"D1m{g}", name=f"D1m{g}_{k}")
                D2m = tmp_pool.tile([P, FG], DT, tag=f"D2m{g}", name=f"D2m{g}_{k}")

                nc.vector.tensor_tensor(o_cp, u_s, o_pi, OP.is_ge)
                nc.vector.tensor_scalar(o_s1, T1, c21, c01, OP.mult, OP.add)
                nc.gpsimd.tensor_scalar(o_s2, T2, c22, c02, OP.mult, OP.add)
                nc.vector.tensor_tensor(m1[:], o_s1, n_s, OP.mult)
                nc.gpsimd.tensor_tensor(m2[:], o_s2, n_s, OP.mult)
                # pre-add mu into the branch bases; d outputs are batched later
                nc.gpsimd.tensor_scalar(D1m[:], mu, 1.0 + c_mu, D1b,
                                        OP.mult, OP.add)
                nc.gpsimd.tensor_scalar(D2m[:], mu, 1.0 + j_mu, D2b,
                                        OP.mult, OP.add)
                nc.vector.tensor_tensor(e1[:], m1[:], D1m[:], OP.add)
                nc.gpsimd.tensor_tensor(e2[:], m2[:], D2m[:], OP.add)
                nc.vector.copy_predicated(
                    e1[:], o_cp.bitcast(mybir.dt.uint32), e2[:])
                nc.vector.tensor_scalar(o_mu, e1[:], MU_MIN, MU_MAX,
                                        OP.max, OP.min)
                mu_g[g] = o_mu

            def finish_block(blk, r0=0, r1=K_BLK):
                """batched d1/d2 writeback over stored mu history + out DMA
                for rows [r0:r1] of the block."""
                t0 = blk * K_BLK
                yb = yt[blk]
                lo = max(r0, 1)
                if r1 > lo:
                    nc.vector.tensor_scalar(yb[:, lo:r1, :, 3],
                                            yb[:, lo - 1:r1 - 1, :, 0],
                                            c_mu, D1b, OP.mult, OP.add)
                    nc.vector.tensor_scalar(yb[:, lo:r1, :, 5],
                                            yb[:, lo - 1:r1 - 1, :, 0],
                                            j_mu, D2b, OP.mult, OP.add)
                if r0 == 0:
                    if blk == 0:
                        mu_prev = mu_init[:]
                    else:
                        mu_prev = yt[blk - 1][:, K_BLK - 1, :, 0]
                    nc.vector.tensor_scalar(yb[:, 0, :, 3], mu_prev,
                                            c_mu, D1b, OP.mult, OP.add)
                    nc.vector.tensor_scalar(yb[:, 0, :, 5], mu_prev,
                                            j_mu, D2b, OP.mult, OP.add)
                nc.sync.dma_start(out=y_v[:, t0 + r0:t0 + r1, :, :],
                                  in_=yb[:, r0:r1, :, :])

            issue_in(0)
            for k in range(N_CYCLES):
                blk = k // K_BLK
                if k % K_BLK == 0:
                    yt[blk] = io_pool.tile([P, K_BLK, F, NCH], DT, tag="y",
                                           name=f"y{blk}")
                    if blk + 1 < NBLK:
                        issue_in(blk + 1)
                S1(0, k)
                if k > 0:
                    S2(1, k - 1)
                    if k % K_BLK == K_BLK // 2 + 1:
                        # drain this block's first half early: its d-rows and
                        # output DMA overlap the second half's compute
                        finish_block(blk, 0, K_BLK // 2)
                    if k % K_BLK == 0:
                        finish_block(blk - 1, K_BLK // 2, K_BLK)
                    if k == N_CYCLES - 2:
                        finish_block(NBLK - 1, K_BLK // 2, K_BLK - 2)
                    if k == N_CYCLES - 1:
                        finish_block(NBLK - 1, K_BLK - 2, K_BLK - 1)
                S1(1, k)
                S2(0, k)
            S2(1, N_CYCLES - 1)
            finish_block(NBLK - 1, K_BLK - 1, K_BLK)

    return nc


_CACHE = {}


def _get_nc(consts):
    key = tuple(np.float64(consts).tobytes())
    if key not in _CACHE:
        nc = _build_nc(consts)
        nc.finalize()
        _CACHE[key] = nc
    return _CACHE[key]


def kernel(params, T, u, noise):
    params = np.asarray(params, dtype=np.float32)
    u = np.ascontiguousarray(np.asarray(u, dtype=np.float32))
    noise = np.ascontiguousarray(np.asarray(noise, dtype=np.float32))
    consts = _prep_consts(params, float(np.asarray(T)))
    nc = _get_nc(consts)

    in_maps = []
    for c in range(N_CORES):
        sl = slice(c * B_SH, (c + 1) * B_SH)
        in_maps.append({
            "u": np.ascontiguousarray(u[:, sl]),
            "noise": np.ascontiguousarray(noise[:, sl]),
        })
    res = run_bass_kernel_spmd(nc, in_maps, list(range(N_CORES)))
    out = np.empty((NCH, N_CYCLES, BATCH), np.float32)
    for c in range(N_CORES):
        sl = slice(c * B_SH, (c + 1) * B_SH)
        out[:, :, sl] = res.results[c]["y"].transpose(2, 0, 1)
    return out


if __name__ == "__main__":
    rng = np.random.default_rng(0)
    params = np.array([2.0, -0.1, -1.0, 0.5, 0.01, -0.02, 0.001, -3.0, 1.0, 0.1,
                       0.5, -1.0, 0.02, -1.5, 0.5, 0.12, 0.005], np.float32)
    u = rng.random((N_CYCLES, BATCH), dtype=np.float32)
    noise = rng.standard_normal((N_CYCLES, BATCH), dtype=np.float32)
    y = kernel(params=params, T=np.float32(200.0), u=u, noise=noise)
    print("out", y.shape, y.dtype, float(y[0].mean()))
